# revision 14
# baseline (speedup 1.0000x reference)
"""Trainium2 Bass kernel for nn_Encoder (pre-norm transformer block, LN over
sequence axis) distributed over 8 NeuronCores.

v2: fp8e4m3 DoubleRow matmuls for QKV and attn@V (2x PE rate), merged LN1
stats AllGather, 2048-wide exp, fp8 ReduceScatter, b2/bo folded into xtok.

Scales: x*16 (fp8), W{q,k,v}*16 -> q,k at 256x (bf16), v/32 -> vq at 8x
(fp8), exp scale /65536, p unscaled (fp8), attnT 8x (bf16), Wo*256 host ->
delta psum 2048x -> fp8 RS, y += rs/2048. FFN stays bf16 (fp8 error too big).
"""

import numpy as np
import ml_dtypes
from contextlib import ExitStack

from concourse import bacc, bass_utils
import concourse.bass as bass
import concourse.tile as tile
import concourse.mybir as mybir
from concourse.masks import make_identity

FP32 = mybir.dt.float32
BF16 = mybir.dt.bfloat16
F8 = mybir.dt.float8e4
AF = mybir.ActivationFunctionType
ALU = mybir.AluOpType
AX = mybir.AxisListType
DRM = mybir.MatmulPerfMode.DoubleRow

B, T, C, H, HS = 2, 2048, 1024, 16, 64
NCORE, P = 8, 128
TN = B * T            # 4096 flat tokens
TOK = TN // NCORE     # 512 tokens per core (256 per batch)
TB = TOK // B         # 256 tokens per batch per core
F = 4 * C             # 4096
KK = C // P           # 8 k-tiles over C
M = F // P            # 32 m-blocks over F
MH = M // 2           # resident half of W1
TQ = 1024             # q-column block per attention region (T // 2)
EPS = 1e-5
SC_EXP = float(HS) ** -0.5 / 65536.0
RG = [list(range(NCORE))]

_cache = {}


def build():
    nc = bacc.Bacc("TRN2", target_bir_lowering=False, debug=False,
                   num_devices=NCORE)

    def EIN(name, shape, dtype):
        return nc.dram_tensor(name, shape, dtype, kind="ExternalInput")

    xt = EIN("xt", [C, TN], F8)            # 16*x^T full (replicated)
    xmine = EIN("xmine", [P, TN], F8)      # my 128 channels of 16*x^T
    xtok = EIN("xtok", [TOK, C], FP32)     # my token rows, +bo+b2 folded in
    wq = EIN("wq", [P, KK, P], BF16)       # 16*Wq cat(2 heads) tiled [p,kk,m]
    wk = EIN("wk", [P, KK, P], BF16)
    wv = EIN("wv", [P, KK, P], BF16)
    wor = EIN("wor", [P, C], BF16)         # 256*Wo rows for my heads
    w1t = EIN("w1t", [P, M, KK, P], BF16)  # [p(c in kk), m, kk, mcol]
    w2t = EIN("w2t", [P, M, C], BF16)      # [p(f in q), q, n]
    bq2 = EIN("bq2", [P, 1], FP32)         # 256*bq
    bk2 = EIN("bk2", [P, 1], FP32)         # 256*bk
    bv2 = EIN("bv2", [P, 1], FP32)         # 8*bv
    b1t = EIN("b1t", [P, M], FP32)         # [p, m]
    g1 = EIN("g1", [P, 1], FP32)           # LN1 gamma/beta for my 128 chans
    be1 = EIN("be1", [P, 1], FP32)
    g2f = EIN("g2f", [P, KK], FP32)        # LN2 gamma/beta, all chans (p, kk)
    be2f = EIN("be2f", [P, KK], FP32)
    out = nc.dram_tensor("out", [TOK, C], FP32, kind="ExternalOutput")
    DBG = _cache.get("debug", False)
    if DBG:
        dbg_qT = nc.dram_tensor("dbg_qT", [P, B, T], BF16,
                                kind="ExternalOutput")
        dbg_kT = nc.dram_tensor("dbg_kT", [P, B, T], BF16,
                                kind="ExternalOutput")
        dbg_attnT = nc.dram_tensor("dbg_attnT", [P, TN], BF16,
                                   kind="ExternalOutput")
        dbg_y = nc.dram_tensor("dbg_y", [P, 4, C], FP32,
                               kind="ExternalOutput")
        dbg_h2 = nc.dram_tensor("dbg_h2", [P, KK, TOK], BF16,
                                kind="ExternalOutput")
        dbg_ab = nc.dram_tensor("dbg_ab", [P, KK, 4], FP32,
                                kind="ExternalOutput")

    with tile.TileContext(nc) as tc, ExitStack() as ctx:
        const = ctx.enter_context(tc.tile_pool(name="const", bufs=1))
        dram = ctx.enter_context(tc.tile_pool(name="dram", bufs=1, space="DRAM"))
        persist = ctx.enter_context(tc.tile_pool(name="acts", bufs=1))

        # ---------------- DRAM comm tiles ----------------
        ab_in = dram.tile([P, 4], FP32, name="abi")
        ab_out = dram.tile([NCORE * P, 4], FP32, name="abo")
        rs_in = [dram.tile([T, C], F8, name=f"rsi{b}") for b in range(B)]
        rs_out = [dram.tile([TB, C], F8, name=f"rso{b}") for b in range(B)]
        ag_in = [dram.tile([P, 2 * KK], FP32, name=f"agi{b}") for b in range(B)]
        ag_out = [dram.tile([NCORE * P, 2 * KK], FP32, name=f"ago{b}")
                  for b in range(B)]

        with tc.tile_pool(name="attn_acts", bufs=1) as acts, \
             tc.tile_pool(name="ph2l", bufs=4) as ph2l, \
             tc.tile_pool(name="p8p", bufs=3) as p8p, \
             tc.tile_pool(name="dstg", bufs=3) as dstg:
            qT_sb = acts.tile([P, B, T], BF16)
            kT_sb = acts.tile([P, B, T], BF16)
            # v in fp8, 8x scale: per (b,hd,pair,par): [0:32]=v/32 lo,
            # [32]=ones, [33:65]=v/32 hi, [65] unused
            vq = acts.tile([P, B, 2, KK, 2, 66], F8)
            attnT = acts.tile([P, TN], BF16)

            p1_ctx = ExitStack()
            p1 = p1_ctx.enter_context(tc.tile_pool(name="p1", bufs=1))
            xm_sb = p1.tile([P, TN], F8)
            nc.sync.dma_start(xm_sb[:], xmine.ap())
            xt1_sb = p1.tile([P, KK, T], F8)

            p2_ctx = ExitStack()
            p2 = p2_ctx.enter_context(tc.tile_pool(name="p2", bufs=1))
            xt0_sb = p2.tile([P, KK, T], F8)
            src_v = xt.ap().rearrange("(kk p) n -> p kk n", p=P)
            for kk in range(2):
                nc.gpsimd.dma_start(xt0_sb[:, kk, :], src_v[:, kk, 0:T])

            ident = const.tile([P, P], FP32)
            make_identity(nc, ident)
            ones1 = const.tile([1, P], FP32)
            nc.vector.memset(ones1[:], 1.0)
            onesc_f = const.tile([P, 1], FP32)
            nc.vector.memset(onesc_f[:], 1.0)
            onesc_b = const.tile([P, 1], BF16)
            nc.vector.memset(onesc_b[:], 1.0)

            def ldconst(t, shape, dt=FP32):
                s = const.tile(shape, dt, name=t.name + "_sb")
                nc.sync.dma_start(s[:], t.ap())
                return s

            def declconst(t, shape, dt=FP32):
                return const.tile(shape, dt, name=t.name + "_sb")

            g1_sb = ldconst(g1, [P, 1])
            be1_sb = ldconst(be1, [P, 1])
            wq_sb = declconst(wq, [P, KK, P], BF16)
            wk_sb = declconst(wk, [P, KK, P], BF16)
            wv_sb = declconst(wv, [P, KK, P], BF16)
            wor_sb = declconst(wor, [P, C], BF16)
            bq_sb = declconst(bq2, [P, 1])
            bk_sb = declconst(bk2, [P, 1])
            bv_sb = declconst(bv2, [P, 1])
            b1_sb = declconst(b1t, [P, M])
            g2_sb = declconst(g2f, [P, KK])
            be2_sb = declconst(be2f, [P, KK])

            # long-lived activations
            xtok_sb = persist.tile([P, B * 2, C], FP32)  # my tokens; becomes y
            wqf = [persist.tile([P, KK, P], F8, name=f"wqf{b}")
                   for b in range(B)]
            wkf = [persist.tile([P, KK, P], F8, name=f"wkf{b}")
                   for b in range(B)]
            wvf = [persist.tile([P, KK, P], F8, name=f"wvf{b}")
                   for b in range(B)]
            bqf = [persist.tile([P, 1], FP32, name=f"bqf{b}") for b in range(B)]
            bkf = [persist.tile([P, 1], FP32, name=f"bkf{b}") for b in range(B)]
            cvec = [persist.tile([P, 1], FP32, name=f"cvec{b}")
                    for b in range(B)]
            ab_sb = persist.tile([P, KK, 4], FP32)
            bb_sb = persist.tile([P, KK, 2], BF16)
            ab_loc = persist.tile([P, 4], FP32)

            def ln_stats(b, stats):
                """(A, 16B) for my 128 chans of batch b -> ab_loc[:, 2b:2b+2].
                Input is 16*x in fp8: s1 = 16*sum, s2 = 256*sumsq; work in
                scaled units (mean_s=16*mean, den_s=16*sqrt(var))."""
                n = T
                eng = nc.vector if b == 0 else nc.gpsimd
                xsrc = xm_sb[:, b * T:(b + 1) * T]
                s1 = stats.tile([P, 1], FP32, tag=f"s1{b}")
                s2 = stats.tile([P, 1], FP32, tag=f"s2{b}")
                scr = stats.tile([P, n], FP32, tag=f"scr{b}", bufs=1)
                scr2 = stats.tile([P, n], FP32, tag=f"scs{b}", bufs=1)
                eng.scalar_tensor_tensor(
                    out=scr2[:], in0=xsrc, scalar=1.0, in1=xsrc,
                    op0=ALU.mult, op1=ALU.bypass, accum_out=s1[:])
                eng.scalar_tensor_tensor(
                    out=scr[:], in0=xsrc, scalar=1.0, in1=xsrc,
                    op0=ALU.mult, op1=ALU.mult, accum_out=s2[:])
                mean = stats.tile([P, 1], FP32, tag=f"mean{b}")
                nc.vector.tensor_scalar_mul(mean[:], s1[:], 1.0 / n)
                ss = stats.tile([P, 1], FP32, tag=f"ss{b}")
                nc.vector.tensor_mul(ss[:], s1[:], s1[:])
                var = stats.tile([P, 1], FP32, tag=f"var{b}")
                nc.vector.scalar_tensor_tensor(
                    out=var[:], in0=ss[:], scalar=-1.0 / n, in1=s2[:],
                    op0=ALU.mult, op1=ALU.add)
                nc.vector.tensor_scalar_mul(var[:], var[:], 1.0 / (n - 1))
                den = stats.tile([P, 1], FP32, tag=f"den{b}")
                nc.scalar.sqrt(den[:], var[:])
                nc.vector.tensor_scalar_add(den[:], den[:], 16.0 * EPS)
                rden = stats.tile([P, 1], FP32, tag=f"rden{b}")
                nc.vector.reciprocal(rden[:], den[:])
                # A = 16*g1/(den_s+16eps) (=A_true); bb = 16*be1 - mean_s*A
                ga = stats.tile([P, 1], FP32, tag=f"ga{b}")
                nc.vector.tensor_scalar_mul(ga[:], g1_sb[:], 16.0)
                nc.vector.tensor_mul(ab_loc[:, 2 * b:2 * b + 1], ga[:],
                                     rden[:])
                mA = stats.tile([P, 1], FP32, tag=f"mA{b}")
                nc.vector.tensor_mul(mA[:], mean[:],
                                     ab_loc[:, 2 * b:2 * b + 1])
                be16 = stats.tile([P, 1], FP32, tag=f"be{b}")
                nc.vector.tensor_scalar_mul(be16[:], be1_sb[:], 16.0)
                nc.vector.tensor_sub(ab_loc[:, 2 * b + 1:2 * b + 2],
                                     be16[:], mA[:])

            def fold(b, foldp):
                for wbase, wf in ((wq_sb, wqf), (wk_sb, wkf), (wv_sb, wvf)):
                    for kk in range(KK):
                        eng = nc.vector if kk % 2 == 0 else nc.gpsimd
                        eng.tensor_scalar_mul(
                            wf[b][:, kk, :], wbase[:, kk, :],
                            ab_sb[:, kk, 2 * b:2 * b + 1])
                for wbase, bias, dst, dsc in ((wq_sb, bq_sb, bqf, 1.0),
                                              (wk_sb, bk_sb, bkf, 1.0),
                                              (wv_sb, bv_sb, cvec, 1.0 / 32)):
                    ps = foldp.tile([P, 1], FP32, tag="bf")
                    for kk in range(KK):
                        nc.tensor.matmul(
                            ps[:], lhsT=wbase[:, kk, :],
                            rhs=bb_sb[:, kk, b:b + 1],
                            start=(kk == 0), stop=(kk == KK - 1))
                    if dsc == 1.0:
                        nc.vector.tensor_add(dst[b][:], ps[:], bias[:])
                    else:
                        nc.vector.scalar_tensor_tensor(
                            out=dst[b][:], in0=ps[:], scalar=dsc,
                            in1=bias[:], op0=ALU.mult, op1=ALU.add)

            def qk_item(b, xt_src, pool, wf, bias, dst, j):
                def fq():
                    ps = pool.tile([P, 512], FP32, tag="qk", name="qkps")
                    for h in range(2):
                        for kp in range(KK // 2):
                            nc.tensor.matmul(
                                ps[h * 64:(h + 1) * 64, :],
                                lhsT=wf[b][:, 2 * kp:2 * kp + 2,
                                           h * 64:(h + 1) * 64],
                                rhs=xt_src[:, 2 * kp:2 * kp + 2,
                                           j * 512:(j + 1) * 512],
                                start=(kp == 0), stop=(kp == KK // 2 - 1),
                                perf_mode=DRM)
                    nc.vector.tensor_scalar_add(
                        dst[:, b, j * 512:(j + 1) * 512], ps[:], bias[b][:])
                return fq

            def v_item(b, xt_src, pool, qv):
                """One vq k-chunk QUAD (pairs 2qv, 2qv+1; tokens 4qv*128..)."""
                def fv():
                    for sub in range(4):
                        tt = 4 * qv + sub
                        vps_f = pool.tile([P, 512], FP32, tag="qk",
                                          name="vps")
                        vps = vps_f[:, 0:P]
                        for h in range(2):
                            for kp in range(KK // 2):
                                nc.tensor.matmul(
                                    vps[h * 64:(h + 1) * 64, :],
                                    lhsT=xt_src[:, 2 * kp:2 * kp + 2,
                                                tt * P + h * 64:
                                                tt * P + (h + 1) * 64],
                                    rhs=wvf[b][:, 2 * kp:2 * kp + 2, :],
                                    start=(kp == 0),
                                    stop=(kp == KK // 2 - 1),
                                    perf_mode=DRM)
                        # vchan c=hd*64+half*32+i -> vq[..,hd,pr,par,half*33+i]
                        dst = vq[:, b, :, 2 * qv + sub // 2, sub % 2,
                                 :].rearrange(
                            "p h (w x) -> p h w x", x=33)[:, :, :, 0:32]
                        src = vps.rearrange("p (h w x) -> p h w x", h=2, w=2)
                        nc.vector.tensor_scalar_mul(dst, src, 1.0 / 32)
                return fv

            def attention(b, sp, attp, fill_jq, post_jq=None):
                """Regions (jq, hd) of 512 q-cols; 2048-wide exp over k-quads;
                DoubleRow attn@V with ones-row denominators in separate psum
                banks; normalize into attnT (8x scale)."""
                for jq in range(4):
                    fill = fill_jq[jq] or []
                    for hd in range(2):
                        att = attp.tile([P, 2, 512], FP32, tag="att",
                                        name=f"att{b}{jq}{hd}")
                        for qd in range(4):
                            s_ps = sp.tile([P, 4, 512], FP32, tag="s")
                            for kc in range(4):
                                k0 = qd * 4 + kc
                                nc.tensor.matmul(
                                    s_ps[:, kc, :],
                                    lhsT=kT_sb[hd * 64:(hd + 1) * 64, b,
                                               k0 * P:(k0 + 1) * P],
                                    rhs=qT_sb[hd * 64:(hd + 1) * 64, b,
                                              jq * 512:(jq + 1) * 512],
                                    start=True, stop=True)
                            p8t = p8p.tile([P, 4, 512], F8, tag="p8")
                            nc.scalar.activation(p8t[:], s_ps[:], AF.Exp,
                                                 scale=SC_EXP)
                            for pp in range(2):
                                pr = qd * 2 + pp
                                nc.tensor.matmul(
                                    att[0:33, 0, :],
                                    lhsT=vq[:, b, hd, pr, :, 0:33],
                                    rhs=p8t[:, 2 * pp:2 * pp + 2, :],
                                    start=(qd == 0 and pp == 0),
                                    stop=(qd == 3 and pp == 1),
                                    perf_mode=DRM)
                                nc.tensor.matmul(
                                    att[64:96, 1, :],
                                    lhsT=vq[:, b, hd, pr, :, 33:65],
                                    rhs=p8t[:, 2 * pp:2 * pp + 2, :],
                                    start=(qd == 0 and pp == 0),
                                    stop=(qd == 3 and pp == 1),
                                    perf_mode=DRM)
                            if fill:
                                it = fill.pop(0)
                                if it is not None:
                                    it()
                        # normalize: rows 0:32 (v lo), 64:96 (v hi), 32=denom
                        rden = ph2l.tile([1, 512], FP32, tag="rden", bufs=2)
                        nc.vector.reciprocal(rden[:], att[32:33, 0, :])
                        rdf = sp.tile([P, 512], FP32, tag="s", name="rdf")
                        nc.tensor.matmul(rdf[0:64, :], lhsT=ones1[:, 0:64],
                                         rhs=rden[:], start=True, stop=True)
                        base = b * T + jq * 512
                        nc.vector.tensor_mul(
                            attnT[hd * 64:hd * 64 + 32, base:base + 512],
                            att[0:32, 0, :], rdf[0:32, :])
                        nc.vector.tensor_mul(
                            attnT[hd * 64 + 32:hd * 64 + 64, base:base + 512],
                            att[64:96, 1, :], rdf[32:64, :])
                    base = b * T + jq * 512
                    nc.vector.tensor_scalar_add(
                        attnT[:, base:base + 512],
                        attnT[:, base:base + 512], cvec[b][:])
                    if post_jq is not None:
                        post_jq(jq)
                    while fill:
                        it = fill.pop(0)
                        if it is not None:
                            it()

            def delta_items(b, sp, jq):
                """4 token-chunk items of delta for q-quarter jq; fp8 out at
                2048x scale."""
                items = []
                for ci in range(4):
                    def fd(ci=ci):
                        tc_i = jq * 4 + ci
                        d_sb = dstg.tile([P, C], F8, tag="dsb", bufs=3)
                        for nh in range(2):
                            dps = sp.tile([P, 512], FP32, tag="qk",
                                          name="dps")
                            nc.tensor.matmul(
                                dps[:],
                                lhsT=attnT[:, b * T + tc_i * P:
                                           b * T + (tc_i + 1) * P],
                                rhs=wor_sb[:, nh * 512:(nh + 1) * 512],
                                start=True, stop=True)
                            sl = d_sb[:, nh * 512:(nh + 1) * 512]
                            if nh == 0:
                                nc.vector.tensor_copy(sl, dps[:])
                            else:
                                nc.gpsimd.tensor_copy(sl, dps[:])
                        nc.sync.dma_start(
                            rs_in[b][tc_i * P:(tc_i + 1) * P, :], d_sb[:])
                    items.append(fd)
                return items

            def rs_go(b):
                nc.gpsimd.collective_compute(
                    "ReduceScatter", ALU.add, replica_groups=RG,
                    ins=[rs_in[b].opt()], outs=[rs_out[b].opt()])

            # ================= phase A: stats, folds, QKV(0) ===============
            with tc.tile_pool(name="stats", bufs=2) as stats, \
                 tc.tile_pool(name="foldp", bufs=2, space="PSUM") as foldp, \
                 tc.tile_pool(name="qkp", bufs=4, space="PSUM") as qkp:
                ln_stats(0, stats)
                ln_stats(1, stats)
                nc.scalar.dma_start(ab_in[:], ab_loc[:])
                nc.gpsimd.collective_compute(
                    "AllGather", ALU.bypass, replica_groups=RG,
                    ins=[ab_in.opt()], outs=[ab_out.opt()])
                # bulk loads queued AFTER the tiny stats DMA so the
                # AllGather isn't stuck behind them on the DMA engines
                for kk in range(2, KK):
                    nc.sync.dma_start(xt0_sb[:, kk, :], src_v[:, kk, 0:T])
                for wsb, wt in ((wq_sb, wq), (wk_sb, wk), (wv_sb, wv)):
                    nc.sync.dma_start(wsb[:], wt.ap())
                nc.sync.dma_start(bq_sb[:], bq2.ap())
                nc.sync.dma_start(bk_sb[:], bk2.ap())
                nc.sync.dma_start(bv_sb[:], bv2.ap())
                for kk in range(KK):
                    nc.gpsimd.dma_start(xt1_sb[:, kk, :], src_v[:, kk, T:TN])
                nc.sync.dma_start(wor_sb[:], wor.ap())
                nc.sync.dma_start(b1_sb[:], b1t.ap())
                nc.sync.dma_start(g2_sb[:], g2f.ap())
                nc.sync.dma_start(be2_sb[:], be2f.ap())
                nc.sync.dma_start(
                    ab_sb[:], ab_out.rearrange("(kk p) s -> p kk s", p=P))
                nc.vector.tensor_copy(bb_sb[:, :, 0], ab_sb[:, :, 1])
                nc.vector.tensor_copy(bb_sb[:, :, 1], ab_sb[:, :, 3])
                nc.vector.memset(vq[:, :, :, :, :, 32:33], 1.0)
                fold(0, foldp)
                fold(1, foldp)
                # serial prefix of QKV(0): K all j, Q j0, V quad 0
                for j in range(4):
                    qk_item(0, xt0_sb, qkp, wkf, bkf, kT_sb, j)()
                qk_item(0, xt0_sb, qkp, wqf, bqf, qT_sb, 0)()
                v_item(0, xt0_sb, qkp, 0)()

            # ====== phase B: attention(0) + QKV(0/1) fill + delta/RS(0) ====
            with tc.tile_pool(name="sp0", bufs=1, space="PSUM") as sp0, \
                 tc.tile_pool(name="qk0", bufs=2, space="PSUM") as qk0, \
                 tc.tile_pool(name="attp0", bufs=1, space="PSUM") as attp0:
                # V quads must stay one fill-slot ahead of the attn quad
                # that consumes them; QKV(1) spreads over later regions
                # interleaved with delta(0) chunks.
                vq0 = [v_item(0, xt0_sb, qk0, qv) for qv in range(1, 4)]
                q0r = [qk_item(0, xt0_sb, qk0, wqf, bqf, qT_sb, j)
                       for j in (1, 2, 3)]
                k1 = [qk_item(1, xt1_sb, qk0, wkf, bkf, kT_sb, j)
                      for j in range(4)]
                q1 = [qk_item(1, xt1_sb, qk0, wqf, bqf, qT_sb, j)
                      for j in range(4)]
                v1 = [v_item(1, xt1_sb, qk0, qv) for qv in range(4)]
                dd = [delta_items(0, qk0, jq) for jq in range(4)]
                fills = [
                    vq0 + q0r + [None, None],              # jq0
                    k1 + dd[0],                            # jq1
                    q1 + dd[1],                            # jq2
                    v1 + dd[2],                            # jq3
                ]

                def post_jq0(jq):
                    if jq == 3:
                        for it in dd[3]:
                            it()
                        rs_go(0)
                attention(0, sp0, attp0, fills, post_jq=post_jq0)
            p2_ctx.close()    # free xt0
            p1_ctx.close()    # free xm + xt1

            # late pools reuse that SBUF
            late_ctx = ExitStack()
            w1res = late_ctx.enter_context(tc.tile_pool(name="w1res", bufs=1))
            tailp = late_ctx.enter_context(tc.tile_pool(name="tail", bufs=1))
            w1a = w1res.tile([P, MH, KK, P], BF16)
            nc.sync.dma_start(w1a[:, 0:MH // 2, :, :],
                              w1t.ap()[:, 0:MH // 2, :, :])
            nc.gpsimd.dma_start(w1a[:, MH // 2:MH, :, :],
                                w1t.ap()[:, MH // 2:MH, :, :])
            nc.sync.dma_start(
                xtok_sb[:], xtok.ap().rearrange("(tc p) c -> p tc c", p=P))
            yT = tailp.tile([P, KK, TOK], FP32)
            h2T = tailp.tile([P, KK, TOK], BF16)
            uT = tailp.tile([P, M, TOK], BF16)

            with tc.tile_pool(name="ph3l", bufs=1) as ph3l, \
                 tc.tile_pool(name="st2", bufs=2) as st2, \
                 tc.tile_pool(name="ffnl", bufs=3) as ffnl, \
                 tc.tile_pool(name="ffno", bufs=2) as ffno:

                def ph3_prep_items(b, stpool):
                    """y = x + rs/2048; per-channel (sum, sumsq) partials via
                    ones-column matmuls on token-major y -> AllGather."""
                    y2 = [st2.tile([P, C], BF16, tag=f"y2{j}", bufs=1,
                                   name=f"y2_{b}{j}") for j in range(2)]

                    def f1():
                        dtok = ph3l.tile([P, 2, C], F8, tag="dtok")
                        nc.gpsimd.dma_start(
                            dtok[:],
                            rs_out[b].rearrange("(j p) c -> p j c", p=P))
                        for j in range(2):
                            nc.gpsimd.scalar_tensor_tensor(
                                out=xtok_sb[:, b * 2 + j, :],
                                in0=dtok[:, j, :], scalar=1.0 / 2048,
                                in1=xtok_sb[:, b * 2 + j, :],
                                op0=ALU.mult, op1=ALU.add)
                            nc.vector.tensor_mul(
                                y2[j][:], xtok_sb[:, b * 2 + j, :],
                                xtok_sb[:, b * 2 + j, :])

                    def f2():
                        stps_f = stpool.tile([P, 512], FP32, tag="qk",
                                             name="stps")
                        stps = stps_f[:, 0:4 * KK]
                        for cc in range(KK):
                            for j in range(2):
                                nc.tensor.matmul(
                                    stps[:, 4 * cc + j:4 * cc + j + 1],
                                    lhsT=xtok_sb[:, b * 2 + j,
                                                 cc * P:(cc + 1) * P],
                                    rhs=onesc_f[:], start=True, stop=True)
                                nc.tensor.matmul(
                                    stps[:, 4 * cc + 2 + j:4 * cc + 3 + j],
                                    lhsT=y2[j][:, cc * P:(cc + 1) * P],
                                    rhs=onesc_b[:], start=True, stop=True)
                        sts = st2.tile([P, 4 * KK], FP32, tag="sts")
                        nc.vector.tensor_copy(sts[:], stps[:])
                        st = st2.tile([P, 2 * KK], FP32, tag="st")
                        sv = sts.rearrange("p (k j) -> p k j", j=2)
                        nc.vector.tensor_add(st[:], sv[:, :, 0], sv[:, :, 1])
                        nc.scalar.dma_start(ag_in[b][:], st[:])
                        nc.gpsimd.collective_compute(
                            "AllGather", ALU.bypass, replica_groups=RG,
                            ins=[ag_in[b].opt()], outs=[ag_out[b].opt()])
                    return [f1, f2]

                def ph3_prep(b, stpool):
                    for f in ph3_prep_items(b, stpool):
                        f()

                def ph3_transpose_items(b, tpp):
                    items = []
                    for j in range(2):
                        for cc in range(KK):
                            def ft(j=j, cc=cc):
                                tp_f = tpp.tile([P, 512], FP32, tag="qk",
                                                name="tp")
                                tp = tp_f[:, 0:P]
                                nc.tensor.transpose(
                                    tp,
                                    xtok_sb[:, b * 2 + j,
                                            cc * P:(cc + 1) * P],
                                    ident[:])
                                nc.vector.tensor_copy(
                                    yT[:, cc, b * TB + j * P:
                                       b * TB + (j + 1) * P], tp)
                            items.append(ft)
                    return items

                def ph3_ab2(b):
                    stg = st2.tile([P, NCORE, 2 * KK], FP32, tag="stg")
                    nc.gpsimd.dma_start(
                        stg[:], ag_out[b].rearrange("(r p) s -> p r s", p=P))
                    for step in (4, 2, 1):
                        nc.vector.tensor_add(
                            stg[:, 0:step, :], stg[:, 0:step, :],
                            stg[:, step:2 * step, :])
                    stf = stg[:, 0, :].rearrange("p (k s) -> p k s", s=2)
                    mean2 = st2.tile([P, KK], FP32, tag="mean2")
                    nc.vector.tensor_scalar_mul(mean2[:], stf[:, :, 0],
                                                1.0 / T)
                    ss2 = st2.tile([P, KK], FP32, tag="ss2")
                    nc.vector.tensor_mul(ss2[:], stf[:, :, 0], stf[:, :, 0])
                    var2 = st2.tile([P, KK], FP32, tag="var2")
                    nc.vector.scalar_tensor_tensor(
                        out=var2[:], in0=ss2[:], scalar=-1.0 / T,
                        in1=stf[:, :, 1], op0=ALU.mult, op1=ALU.add)
                    nc.vector.tensor_scalar_mul(var2[:], var2[:],
                                                1.0 / (T - 1))
                    den2 = st2.tile([P, KK], FP32, tag="den2")
                    nc.scalar.sqrt(den2[:], var2[:])
                    nc.vector.tensor_scalar_add(den2[:], den2[:], EPS)
                    rden2 = st2.tile([P, KK], FP32, tag="rden2")
                    nc.vector.reciprocal(rden2[:], den2[:])
                    A2 = st2.tile([P, KK], FP32, tag="A2")
                    nc.vector.tensor_mul(A2[:], g2_sb[:], rden2[:])
                    mA2 = st2.tile([P, KK], FP32, tag="mA2")
                    nc.vector.tensor_mul(mA2[:], mean2[:], A2[:])
                    B2 = st2.tile([P, KK], FP32, tag="B2")
                    nc.vector.tensor_sub(B2[:], be2_sb[:], mA2[:])
                    return A2, B2

                def ph3_h2(b, ab2):
                    A2, B2 = ab2
                    for kk in range(KK):
                        nc.vector.tensor_scalar(
                            out=h2T[:, kk, b * TB:(b + 1) * TB],
                            in0=yT[:, kk, b * TB:(b + 1) * TB],
                            scalar1=A2[:, kk:kk + 1],
                            scalar2=B2[:, kk:kk + 1],
                            op0=ALU.mult, op1=ALU.add)

                def ph3_finish(b):
                    ph3_h2(b, ph3_ab2(b))

                def ffn_w1_items(b, up):
                    items = []
                    for m in range(M):
                        def fm(m=m):
                            if m < MH:
                                w1_sl = w1a[:, m, :, :]
                            else:
                                w1_t = ffnl.tile([P, KK, P], BF16, tag="w1",
                                                 bufs=3)
                                nc.sync.dma_start(w1_t[:],
                                                  w1t.ap()[:, m, :, :])
                                w1_sl = w1_t[:]
                            ups = up.tile([P, TB], FP32, tag="qk", name="u")
                            for kk in range(KK):
                                nc.tensor.matmul(
                                    ups[:], lhsT=w1_sl[:, kk, :],
                                    rhs=h2T[:, kk, b * TB:(b + 1) * TB],
                                    start=(kk == 0), stop=(kk == KK - 1))
                            nc.scalar.activation(
                                uT[:, m, b * TB:(b + 1) * TB], ups[:],
                                AF.Relu, bias=b1_sb[:, m:m + 1], scale=1.0)
                        items.append(fm)
                    return items

                def ffn_w2(b, zp, mid_cb=None):
                    zt = [zp.tile([P, C], FP32, tag="z", name=f"z{b}{j}")
                          for j in range(2)]
                    for q in range(M):
                        if q == M // 2 and mid_cb is not None:
                            mid_cb()
                        w2_sl = ffnl.tile([P, C], BF16, tag="w2", bufs=3)
                        nc.sync.dma_start(w2_sl[:], w2t.ap()[:, q, :])
                        for j in range(2):
                            for nh in range(2):
                                nc.tensor.matmul(
                                    zt[j][:, nh * 512:(nh + 1) * 512],
                                    lhsT=uT[:, q, b * TB + j * P:
                                            b * TB + (j + 1) * P],
                                    rhs=w2_sl[:, nh * 512:(nh + 1) * 512],
                                    start=(q == 0), stop=(q == M - 1))
                    for j in range(2):
                        tc_i = b * 2 + j
                        o_sb = ffno.tile([P, C], FP32, tag="o", bufs=2)
                        nc.vector.tensor_add(o_sb[:], zt[j][:],
                                             xtok_sb[:, tc_i, :])
                        nc.sync.dma_start(
                            out.ap()[tc_i * P:(tc_i + 1) * P, :], o_sb[:])

                # ========== phase C: attention(1) + ph3(0) fill ==========
                with tc.tile_pool(name="sp1", bufs=1, space="PSUM") as sp1, \
                     tc.tile_pool(name="qk1", bufs=2, space="PSUM") as qk1, \
                     tc.tile_pool(name="attp1", bufs=1, space="PSUM") as attp1:
                    pits = ph3_prep_items(0, qk1)
                    tps = ph3_transpose_items(0, qk1)
                    ddc = [delta_items(1, qk1, jq) for jq in range(4)]
                    w1_0 = ffn_w1_items(0, qk1)
                    # f1 needs RS(0) done (~21us after phase-B end): safe in
                    # late jq1 slots; f2 (stats + AG2(0) issue) right after.
                    fill_c = [
                        [None] * 8,                                    # jq0
                        ddc[0] + [None, pits[0], None, pits[1]],       # jq1
                        ddc[1] + tps[0:4],                             # jq2
                        ddc[2] + tps[4:8],                             # jq3
                    ]

                    def post_jq1(jq):
                        if jq == 3:
                            for it in ddc[3]:
                                it()
                            rs_go(1)
                    attention(1, sp1, attp1, fill_c, post_jq=post_jq1)
                    for it in tps[8:16]:
                        it()
                    ph3_h2(0, ph3_ab2(0))
                    for it in w1_0[0:16]:
                        it()

                # ================= phase D: FFN + ph3(1) =================
                with tc.tile_pool(name="ffp", bufs=2, space="PSUM") as up:
                    for it in ffn_w1_items(0, up)[16:]:
                        it()
                with tc.tile_pool(name="tpp2b", bufs=2, space="PSUM") as t2b, \
                     tc.tile_pool(name="zp0", bufs=2, space="PSUM") as zp0:
                    def mid():
                        ph3_prep(1, t2b)
                        for it in ph3_transpose_items(1, t2b):
                            it()
                    ffn_w2(0, zp0, mid_cb=mid)
                    ph3_finish(1)
                with tc.tile_pool(name="ffp1", bufs=2, space="PSUM") as up1:
                    for it in ffn_w1_items(1, up1):
                        it()
                with tc.tile_pool(name="zp1", bufs=2, space="PSUM") as zp1:
                    ffn_w2(1, zp1)
                if DBG:
                    nc.sync.dma_start(dbg_qT.ap(), qT_sb[:])
                    nc.sync.dma_start(dbg_kT.ap(), kT_sb[:])
                    nc.sync.dma_start(dbg_attnT.ap(), attnT[:])
                    nc.sync.dma_start(dbg_y.ap(), xtok_sb[:])
                    nc.sync.dma_start(dbg_h2.ap(), h2T[:])
                    nc.sync.dma_start(dbg_ab.ap(), ab_sb[:])
            late_ctx.close()

    nc.compile()
    return nc


def prep_inputs(x, Wq, bq, Wk, bk, Wv, bv, Wo, bo, W1, b1, W2, b2,
                gamma1, beta1, gamma2, beta2):
    bf = ml_dtypes.bfloat16
    f8 = ml_dtypes.float8_e4m3
    xf = np.asarray(x, np.float32).reshape(TN, C)
    xt_full = np.ascontiguousarray(xf.T * 16.0).astype(f8)       # [C, TN]
    w1_full = np.ascontiguousarray(
        np.asarray(W1, np.float32).reshape(KK, P, M, P)
        .transpose(1, 2, 0, 3)).astype(bf)                       # [P, M, KK, P]
    w2_full = np.ascontiguousarray(
        np.asarray(W2, np.float32).reshape(M, P, C)
        .transpose(1, 0, 2)).astype(bf)                          # [P, M, C]
    b1_t = np.ascontiguousarray(b1.reshape(M, P).T).astype(np.float32)
    g2t = np.ascontiguousarray(gamma2.reshape(KK, P).T).astype(np.float32)
    be2t = np.ascontiguousarray(beta2.reshape(KK, P).T).astype(np.float32)
    bob2 = (np.asarray(bo, np.float32) + np.asarray(b2, np.float32))[None, :]

    in_maps = []
    for i in range(NCORE):
        ci = slice(P * i, P * (i + 1))
        hA, hB = 2 * i, 2 * i + 1

        def tile_km(wcat):  # [C, 128] -> [p, kk, m], 16x scale
            return np.ascontiguousarray(
                (wcat * 16.0).reshape(KK, P, P).transpose(1, 0, 2)).astype(bf)

        wq_cat = np.concatenate([Wq[hA], Wq[hB]], axis=1)
        wk_cat = np.concatenate([Wk[hA], Wk[hB]], axis=1)
        wv_cat = np.concatenate([Wv[hA], Wv[hB]], axis=1)
        xtok_i = np.concatenate(
            [xf[i * TB:(i + 1) * TB], xf[T + i * TB:T + (i + 1) * TB]],
            axis=0) + bob2
        in_maps.append({
            "xt": xt_full,
            "xmine": np.ascontiguousarray(xt_full[ci]),
            "xtok": np.ascontiguousarray(xtok_i.astype(np.float32)),
            "wq": tile_km(np.asarray(wq_cat, np.float32)),
            "wk": tile_km(np.asarray(wk_cat, np.float32)),
            "wv": tile_km(np.asarray(wv_cat, np.float32)),
            "wor": np.ascontiguousarray(
                np.asarray(Wo, np.float32)[ci] * 256.0).astype(bf),
            "w1t": w1_full,
            "w2t": w2_full,
            "bq2": (np.concatenate([bq[hA], bq[hB]])[:, None]
                    * 256.0).astype(np.float32),
            "bk2": (np.concatenate([bk[hA], bk[hB]])[:, None]
                    * 256.0).astype(np.float32),
            "bv2": (np.concatenate([bv[hA], bv[hB]])[:, None]
                    * 8.0).astype(np.float32),
            "b1t": b1_t,
            "g1": gamma1[ci][:, None].astype(np.float32),
            "be1": beta1[ci][:, None].astype(np.float32),
            "g2f": g2t,
            "be2f": be2t,
        })
    return in_maps


def kernel(**inputs):
    inputs = {k: np.asarray(v) for k, v in inputs.items()}
    if "nc" not in _cache:
        _cache["nc"] = build()
    nc = _cache["nc"]
    in_maps = prep_inputs(**inputs)
    res = bass_utils.run_bass_kernel_spmd(nc, in_maps,
                                          core_ids=list(range(NCORE)))
    outf = np.zeros((TN, C), np.float32)
    for i in range(NCORE):
        o = res.results[i]["out"]
        outf[i * TB:(i + 1) * TB] = o[0:TB]
        outf[T + i * TB:T + (i + 1) * TB] = o[TB:TOK]
    return outf.reshape(B, T, C).astype(np.float32)


# revision 21
# speedup vs baseline: 1.1787x; 1.1787x over previous
"""Trainium2 Bass kernel for nn_Encoder (pre-norm transformer block, LN over
sequence axis) distributed over 8 NeuronCores.

v2: fp8e4m3 DoubleRow matmuls for QKV and attn@V (2x PE rate), merged LN1
stats AllGather, 2048-wide exp, fp8 ReduceScatter, b2/bo folded into xtok.

Scales: x*16 (fp8), W{q,k,v}*16 -> q,k at 256x (bf16), v/32 -> vq at 8x
(fp8), exp scale /65536, p unscaled (fp8), attnT 8x (bf16), Wo*256 host ->
delta psum 2048x -> fp8 RS, y += rs/2048. FFN stays bf16 (fp8 error too big).
"""

import numpy as np
import ml_dtypes
from contextlib import ExitStack

from concourse import bacc, bass_utils
import concourse.bass as bass
import concourse.tile as tile
import concourse.mybir as mybir
from concourse.masks import make_identity

FP32 = mybir.dt.float32
BF16 = mybir.dt.bfloat16
F8 = mybir.dt.float8e4
AF = mybir.ActivationFunctionType
ALU = mybir.AluOpType
AX = mybir.AxisListType
DRM = mybir.MatmulPerfMode.DoubleRow

B, T, C, H, HS = 2, 2048, 1024, 16, 64
NCORE, P = 8, 128
TN = B * T            # 4096 flat tokens
TOK = TN // NCORE     # 512 tokens per core (256 per batch)
TB = TOK // B         # 256 tokens per batch per core
F = 4 * C             # 4096
KK = C // P           # 8 k-tiles over C
M = F // P            # 32 m-blocks over F
MH = M // 2           # resident half of W1
TQ = 1024             # q-column block per attention region (T // 2)
EPS = 1e-5
SC_EXP = float(HS) ** -0.5 / 65536.0
RG = [list(range(NCORE))]

_cache = {}


def build():
    nc = bacc.Bacc("TRN2", target_bir_lowering=False, debug=False,
                   num_devices=NCORE)

    def EIN(name, shape, dtype):
        return nc.dram_tensor(name, shape, dtype, kind="ExternalInput")

    xt = EIN("xt", [C, TN], F8)            # 16*x^T full (replicated)
    xmine = EIN("xmine", [P, TN], F8)      # my 128 channels of 16*x^T
    xtok = EIN("xtok", [TOK, C], FP32)     # my token rows, +bo+b2 folded in
    wq = EIN("wq", [P, KK, P], BF16)       # 16*Wq cat(2 heads) tiled [p,kk,m]
    wk = EIN("wk", [P, KK, P], BF16)
    wv = EIN("wv", [P, KK, P], BF16)
    wor = EIN("wor", [P, C], BF16)         # 256*Wo rows for my heads
    w1t = EIN("w1t", [P, M, KK, P], BF16)  # [p(c in kk), m, kk, mcol]
    w2t = EIN("w2t", [P, M, C], BF16)      # [p(f in q), q, n]
    bq2 = EIN("bq2", [P, 1], FP32)         # 256*bq
    bk2 = EIN("bk2", [P, 1], FP32)         # 256*bk
    bv2 = EIN("bv2", [P, 1], FP32)         # 8*bv
    b1t = EIN("b1t", [P, M], FP32)         # [p, m]
    g1 = EIN("g1", [P, 1], FP32)           # LN1 gamma/beta for my 128 chans
    be1 = EIN("be1", [P, 1], FP32)
    g2f = EIN("g2f", [P, KK], FP32)        # LN2 gamma/beta, all chans (p, kk)
    be2f = EIN("be2f", [P, KK], FP32)
    out = nc.dram_tensor("out", [TOK, C], FP32, kind="ExternalOutput")
    DBG = _cache.get("debug", False)
    if DBG:
        dbg_qT = nc.dram_tensor("dbg_qT", [P, B, T], BF16,
                                kind="ExternalOutput")
        dbg_kT = nc.dram_tensor("dbg_kT", [P, B, T], BF16,
                                kind="ExternalOutput")
        dbg_attnT = nc.dram_tensor("dbg_attnT", [P, TN], BF16,
                                   kind="ExternalOutput")
        dbg_y = nc.dram_tensor("dbg_y", [P, 4, C], FP32,
                               kind="ExternalOutput")
        dbg_h2 = nc.dram_tensor("dbg_h2", [P, KK, TOK], BF16,
                                kind="ExternalOutput")
        dbg_ab = nc.dram_tensor("dbg_ab", [P, KK, 4], FP32,
                                kind="ExternalOutput")

    with tile.TileContext(nc) as tc, ExitStack() as ctx:
        const = ctx.enter_context(tc.tile_pool(name="const", bufs=1))
        dram = ctx.enter_context(tc.tile_pool(name="dram", bufs=1, space="DRAM"))
        persist = ctx.enter_context(tc.tile_pool(name="acts", bufs=1))

        # ---------------- DRAM comm tiles ----------------
        ab_in = dram.tile([P, 4], FP32, name="abi")
        ab_out = dram.tile([NCORE * P, 4], FP32, name="abo")
        rs_in = [dram.tile([T, C], F8, name=f"rsi{b}") for b in range(B)]
        rs_out = [dram.tile([TB, C], F8, name=f"rso{b}") for b in range(B)]
        ag_in = [dram.tile([P, 2 * KK], FP32, name=f"agi{b}") for b in range(B)]
        ag_out = [dram.tile([NCORE * P, 2 * KK], FP32, name=f"ago{b}")
                  for b in range(B)]

        with tc.tile_pool(name="attn_acts", bufs=1) as acts, \
             tc.tile_pool(name="ph2l", bufs=4) as ph2l, \
             tc.tile_pool(name="p8p", bufs=3) as p8p, \
             tc.tile_pool(name="dstg", bufs=3) as dstg:
            qT_sb = acts.tile([P, B, T], BF16)
            kT_sb = acts.tile([P, B, T], BF16)
            # v in fp8, 8x scale: per (b,hd,pair,par): [0:32]=v/32 lo,
            # [32]=ones, [33:65]=v/32 hi, [65] unused
            vq = acts.tile([P, B, 2, KK, 2, 66], F8)
            attnT = acts.tile([P, TN], BF16)

            p1_ctx = ExitStack()
            p1 = p1_ctx.enter_context(tc.tile_pool(name="p1", bufs=1))
            xm_sb = p1.tile([P, TN], F8)
            nc.sync.dma_start(xm_sb[:], xmine.ap())
            xt1_sb = p1.tile([P, KK, T], F8)

            p2_ctx = ExitStack()
            p2 = p2_ctx.enter_context(tc.tile_pool(name="p2", bufs=1))
            xt0_sb = p2.tile([P, KK, T], F8)
            src_v = xt.ap().rearrange("(kk p) n -> p kk n", p=P)
            for kk in range(2):
                nc.gpsimd.dma_start(xt0_sb[:, kk, :], src_v[:, kk, 0:T])

            ident = const.tile([P, P], FP32)
            make_identity(nc, ident)
            ones1 = const.tile([1, P], FP32)
            nc.vector.memset(ones1[:], 1.0)
            onesc_f = const.tile([P, 1], FP32)
            nc.vector.memset(onesc_f[:], 1.0)
            onesc_b = const.tile([P, 1], BF16)
            nc.vector.memset(onesc_b[:], 1.0)

            def ldconst(t, shape, dt=FP32):
                s = const.tile(shape, dt, name=t.name + "_sb")
                nc.sync.dma_start(s[:], t.ap())
                return s

            def declconst(t, shape, dt=FP32):
                return const.tile(shape, dt, name=t.name + "_sb")

            g1_sb = ldconst(g1, [P, 1])
            be1_sb = ldconst(be1, [P, 1])
            wq_sb = declconst(wq, [P, KK, P], BF16)
            wk_sb = declconst(wk, [P, KK, P], BF16)
            wv_sb = declconst(wv, [P, KK, P], BF16)
            wor_sb = declconst(wor, [P, C], BF16)
            bq_sb = declconst(bq2, [P, 1])
            bk_sb = declconst(bk2, [P, 1])
            bv_sb = declconst(bv2, [P, 1])
            b1_sb = declconst(b1t, [P, M])
            g2_sb = declconst(g2f, [P, KK])
            be2_sb = declconst(be2f, [P, KK])

            # long-lived activations
            xtok_sb = persist.tile([P, B * 2, C], FP32)  # my tokens; becomes y
            wqf = [persist.tile([P, KK, P], F8, name=f"wqf{b}")
                   for b in range(B)]
            wkf = [persist.tile([P, KK, P], F8, name=f"wkf{b}")
                   for b in range(B)]
            wvf = [persist.tile([P, KK, P], F8, name=f"wvf{b}")
                   for b in range(B)]
            bqf = [persist.tile([P, 1], FP32, name=f"bqf{b}") for b in range(B)]
            bkf = [persist.tile([P, 1], FP32, name=f"bkf{b}") for b in range(B)]
            cvec = [persist.tile([P, 1], FP32, name=f"cvec{b}")
                    for b in range(B)]
            ab_sb = persist.tile([P, KK, 4], FP32)
            bb_sb = persist.tile([P, KK, 2], BF16)
            ab_loc = persist.tile([P, 4], FP32)

            def ln_stats(b, stats):
                """(A, 16B) for my 128 chans of batch b -> ab_loc[:, 2b:2b+2].
                Input is 16*x in fp8: s1 = 16*sum, s2 = 256*sumsq; work in
                scaled units (mean_s=16*mean, den_s=16*sqrt(var))."""
                n = T
                eng = nc.vector if b == 0 else nc.gpsimd
                xsrc = xm_sb[:, b * T:(b + 1) * T]
                s1 = stats.tile([P, 1], FP32, tag=f"s1{b}")
                s2 = stats.tile([P, 1], FP32, tag=f"s2{b}")
                scr = stats.tile([P, n], FP32, tag=f"scr{b}", bufs=1)
                scr2 = stats.tile([P, n], FP32, tag=f"scs{b}", bufs=1)
                eng.scalar_tensor_tensor(
                    out=scr2[:], in0=xsrc, scalar=1.0, in1=xsrc,
                    op0=ALU.mult, op1=ALU.bypass, accum_out=s1[:])
                eng.scalar_tensor_tensor(
                    out=scr[:], in0=xsrc, scalar=1.0, in1=xsrc,
                    op0=ALU.mult, op1=ALU.mult, accum_out=s2[:])
                mean = stats.tile([P, 1], FP32, tag=f"mean{b}")
                nc.vector.tensor_scalar_mul(mean[:], s1[:], 1.0 / n)
                ss = stats.tile([P, 1], FP32, tag=f"ss{b}")
                nc.vector.tensor_mul(ss[:], s1[:], s1[:])
                var = stats.tile([P, 1], FP32, tag=f"var{b}")
                nc.vector.scalar_tensor_tensor(
                    out=var[:], in0=ss[:], scalar=-1.0 / n, in1=s2[:],
                    op0=ALU.mult, op1=ALU.add)
                nc.vector.tensor_scalar_mul(var[:], var[:], 1.0 / (n - 1))
                den = stats.tile([P, 1], FP32, tag=f"den{b}")
                nc.scalar.sqrt(den[:], var[:])
                nc.vector.tensor_scalar_add(den[:], den[:], 16.0 * EPS)
                rden = stats.tile([P, 1], FP32, tag=f"rden{b}")
                nc.vector.reciprocal(rden[:], den[:])
                # A = 16*g1/(den_s+16eps) (=A_true); bb = 16*be1 - mean_s*A
                ga = stats.tile([P, 1], FP32, tag=f"ga{b}")
                nc.vector.tensor_scalar_mul(ga[:], g1_sb[:], 16.0)
                nc.vector.tensor_mul(ab_loc[:, 2 * b:2 * b + 1], ga[:],
                                     rden[:])
                mA = stats.tile([P, 1], FP32, tag=f"mA{b}")
                nc.vector.tensor_mul(mA[:], mean[:],
                                     ab_loc[:, 2 * b:2 * b + 1])
                be16 = stats.tile([P, 1], FP32, tag=f"be{b}")
                nc.vector.tensor_scalar_mul(be16[:], be1_sb[:], 16.0)
                nc.vector.tensor_sub(ab_loc[:, 2 * b + 1:2 * b + 2],
                                     be16[:], mA[:])

            def fold(b, foldp):
                for wbase, wf in ((wq_sb, wqf), (wk_sb, wkf), (wv_sb, wvf)):
                    for kk in range(KK):
                        eng = nc.vector if kk % 2 == 0 else nc.gpsimd
                        eng.tensor_scalar_mul(
                            wf[b][:, kk, :], wbase[:, kk, :],
                            ab_sb[:, kk, 2 * b:2 * b + 1])
                for wbase, bias, dst, dsc in ((wq_sb, bq_sb, bqf, 1.0),
                                              (wk_sb, bk_sb, bkf, 1.0),
                                              (wv_sb, bv_sb, cvec, 1.0 / 32)):
                    ps = foldp.tile([P, 1], FP32, tag="bf")
                    for kk in range(KK):
                        nc.tensor.matmul(
                            ps[:], lhsT=wbase[:, kk, :],
                            rhs=bb_sb[:, kk, b:b + 1],
                            start=(kk == 0), stop=(kk == KK - 1))
                    if dsc == 1.0:
                        nc.vector.tensor_add(dst[b][:], ps[:], bias[:])
                    else:
                        nc.vector.scalar_tensor_tensor(
                            out=dst[b][:], in0=ps[:], scalar=dsc,
                            in1=bias[:], op0=ALU.mult, op1=ALU.add)

            def qk_item(b, xt_src, pool, wf, bias, dst, j):
                def fq():
                    ps = pool.tile([P, 512], FP32, tag="qk", name="qkps")
                    for h in range(2):
                        for kp in range(KK // 2):
                            nc.tensor.matmul(
                                ps[h * 64:(h + 1) * 64, :],
                                lhsT=wf[b][:, 2 * kp:2 * kp + 2,
                                           h * 64:(h + 1) * 64],
                                rhs=xt_src[:, 2 * kp:2 * kp + 2,
                                           j * 512:(j + 1) * 512],
                                start=(kp == 0), stop=(kp == KK // 2 - 1),
                                perf_mode=DRM)
                    nc.vector.tensor_scalar_add(
                        dst[:, b, j * 512:(j + 1) * 512], ps[:], bias[b][:])
                return fq

            def v_item(b, xt_src, pool, qv):
                """One vq k-chunk QUAD (pairs 2qv, 2qv+1; tokens 4qv*128..)."""
                def fv():
                    for sub in range(4):
                        tt = 4 * qv + sub
                        vps_f = pool.tile([P, 512], FP32, tag="qk",
                                          name="vps")
                        vps = vps_f[:, 0:P]
                        for h in range(2):
                            for kp in range(KK // 2):
                                nc.tensor.matmul(
                                    vps[h * 64:(h + 1) * 64, :],
                                    lhsT=xt_src[:, 2 * kp:2 * kp + 2,
                                                tt * P + h * 64:
                                                tt * P + (h + 1) * 64],
                                    rhs=wvf[b][:, 2 * kp:2 * kp + 2, :],
                                    start=(kp == 0),
                                    stop=(kp == KK // 2 - 1),
                                    perf_mode=DRM)
                        # vchan c=hd*64+half*32+i -> vq[..,hd,pr,par,half*33+i]
                        dst = vq[:, b, :, 2 * qv + sub // 2, sub % 2,
                                 :].rearrange(
                            "p h (w x) -> p h w x", x=33)[:, :, :, 0:32]
                        src = vps.rearrange("p (h w x) -> p h w x", h=2, w=2)
                        nc.vector.tensor_scalar_mul(dst, src, 1.0 / 32)
                return fv

            def attention(b, sp, attp, qk, fill_jq, post_jq=None):
                """Regions (jq, hd) of 512 q-cols; 1024-wide exp per k-pair,
                double-buffered scores psum; DoubleRow attn@V accumulates
                both v-halves in ONE psum bank (A rows 0:33 with denom,
                B rows 64:96 pre-zeroed, start-group checks skipped);
                normalize deferred into the next region's first slot."""
                pend = [None]

                def norm_epilogue(att, hd, jq):
                    def run():
                        rden = ph2l.tile([1, 512], FP32, tag="rden", bufs=2)
                        nc.vector.reciprocal(rden[:], att[32:33, :])
                        rdf = qk.tile([P, 512], FP32, tag="qk", name="rdf")
                        nc.tensor.matmul(rdf[0:64, :], lhsT=ones1[:, 0:64],
                                         rhs=rden[:], start=True, stop=True)
                        base = b * T + jq * 512
                        nc.vector.tensor_mul(
                            attnT[hd * 64:hd * 64 + 32, base:base + 512],
                            att[0:32, :], rdf[0:32, :])
                        nc.vector.tensor_mul(
                            attnT[hd * 64 + 32:hd * 64 + 64,
                                  base:base + 512],
                            att[64:96, :], rdf[32:64, :])
                        if hd == 1:
                            nc.vector.tensor_scalar_add(
                                attnT[:, base:base + 512],
                                attnT[:, base:base + 512], cvec[b][:])
                            if post_jq is not None:
                                post_jq(jq)
                    return run

                for jq in range(4):
                    fill = fill_jq[jq] or []
                    for hd in range(2):
                        att = attp.tile([P, 512], FP32, tag="att",
                                        name=f"att{b}{jq}{hd}")
                        nc.gpsimd.memset(att[64:96, :], 0.0)
                        for pr in range(KK):
                            s_ps = sp.tile([P, 2, 512], FP32, tag="s")
                            for par in range(2):
                                k0 = pr * 2 + par
                                nc.tensor.matmul(
                                    s_ps[:, par, :],
                                    lhsT=kT_sb[hd * 64:(hd + 1) * 64, b,
                                               k0 * P:(k0 + 1) * P],
                                    rhs=qT_sb[hd * 64:(hd + 1) * 64, b,
                                              jq * 512:(jq + 1) * 512],
                                    start=True, stop=True)
                            p8t = p8p.tile([P, 2, 512], F8, tag="p8")
                            nc.scalar.activation(p8t[:], s_ps[:], AF.Exp,
                                                 scale=SC_EXP)
                            nc.tensor.matmul(
                                att[0:33, :],
                                lhsT=vq[:, b, hd, pr, :, 0:33],
                                rhs=p8t[:], start=(pr == 0),
                                stop=(pr == KK - 1), perf_mode=DRM)
                            nc.tensor.matmul(
                                att[64:96, :],
                                lhsT=vq[:, b, hd, pr, :, 33:65],
                                rhs=p8t[:], start=False, stop=False,
                                skip_group_check=True, perf_mode=DRM)
                            if pend[0] is not None:
                                pend[0]()
                                pend[0] = None
                            elif fill:
                                it = fill.pop(0)
                                if it is not None:
                                    it()
                        pend[0] = norm_epilogue(att, hd, jq)
                    while fill:
                        it = fill.pop(0)
                        if it is not None:
                            it()
                pend[0]()

            def delta_items(b, sp, jq):
                """4 token-chunk items of delta for q-quarter jq; fp8 out at
                2048x scale."""
                items = []
                for ci in range(4):
                    def fd(ci=ci):
                        tc_i = jq * 4 + ci
                        d_sb = dstg.tile([P, C], F8, tag="dsb", bufs=3)
                        for nh in range(2):
                            dps = sp.tile([P, 512], FP32, tag="qk",
                                          name="dps")
                            nc.tensor.matmul(
                                dps[:],
                                lhsT=attnT[:, b * T + tc_i * P:
                                           b * T + (tc_i + 1) * P],
                                rhs=wor_sb[:, nh * 512:(nh + 1) * 512],
                                start=True, stop=True)
                            sl = d_sb[:, nh * 512:(nh + 1) * 512]
                            if nh == 0:
                                nc.vector.tensor_copy(sl, dps[:])
                            else:
                                nc.gpsimd.tensor_copy(sl, dps[:])
                        nc.sync.dma_start(
                            rs_in[b][tc_i * P:(tc_i + 1) * P, :], d_sb[:])
                    items.append(fd)
                return items

            def rs_go(b):
                nc.gpsimd.collective_compute(
                    "ReduceScatter", ALU.add, replica_groups=RG,
                    ins=[rs_in[b].opt()], outs=[rs_out[b].opt()])

            # ================= phase A: stats, folds, QKV(0) ===============
            with tc.tile_pool(name="stats", bufs=2) as stats, \
                 tc.tile_pool(name="foldp", bufs=2, space="PSUM") as foldp, \
                 tc.tile_pool(name="qkp", bufs=4, space="PSUM") as qkp:
                ln_stats(0, stats)
                ln_stats(1, stats)
                nc.scalar.dma_start(ab_in[:], ab_loc[:])
                nc.gpsimd.collective_compute(
                    "AllGather", ALU.bypass, replica_groups=RG,
                    ins=[ab_in.opt()], outs=[ab_out.opt()])
                # bulk loads queued AFTER the tiny stats DMA so the
                # AllGather isn't stuck behind them on the DMA engines
                for kk in range(2, KK):
                    nc.sync.dma_start(xt0_sb[:, kk, :], src_v[:, kk, 0:T])
                for wsb, wt in ((wq_sb, wq), (wk_sb, wk), (wv_sb, wv)):
                    nc.sync.dma_start(wsb[:], wt.ap())
                nc.sync.dma_start(bq_sb[:], bq2.ap())
                nc.sync.dma_start(bk_sb[:], bk2.ap())
                nc.sync.dma_start(bv_sb[:], bv2.ap())
                for kk in range(KK):
                    nc.gpsimd.dma_start(xt1_sb[:, kk, :], src_v[:, kk, T:TN])
                nc.sync.dma_start(wor_sb[:], wor.ap())
                nc.sync.dma_start(b1_sb[:], b1t.ap())
                nc.sync.dma_start(g2_sb[:], g2f.ap())
                nc.sync.dma_start(be2_sb[:], be2f.ap())
                nc.sync.dma_start(
                    ab_sb[:], ab_out.rearrange("(kk p) s -> p kk s", p=P))
                nc.vector.tensor_copy(bb_sb[:, :, 0], ab_sb[:, :, 1])
                nc.vector.tensor_copy(bb_sb[:, :, 1], ab_sb[:, :, 3])
                nc.vector.memset(vq[:, :, :, :, :, 32:33], 1.0)
                fold(0, foldp)
                fold(1, foldp)
                # serial prefix of QKV(0): K all j, Q j0, V quad 0
                for j in range(4):
                    qk_item(0, xt0_sb, qkp, wkf, bkf, kT_sb, j)()
                qk_item(0, xt0_sb, qkp, wqf, bqf, qT_sb, 0)()
                v_item(0, xt0_sb, qkp, 0)()

            # ====== phase B: attention(0) + QKV(0/1) fill + delta/RS(0) ====
            with tc.tile_pool(name="sp0", bufs=2, space="PSUM") as sp0, \
                 tc.tile_pool(name="qk0", bufs=2, space="PSUM") as qk0, \
                 tc.tile_pool(name="attp0", bufs=2, space="PSUM") as attp0:
                # V quads must stay ahead of the attn pair that consumes
                # them; QKV(1) spreads over later regions interleaved with
                # delta(0) chunks. 16 pair-slots per jq (minus norm slots).
                vq0 = [v_item(0, xt0_sb, qk0, qv) for qv in range(1, 4)]
                q0r = [qk_item(0, xt0_sb, qk0, wqf, bqf, qT_sb, j)
                       for j in (1, 2, 3)]
                k1 = [qk_item(1, xt1_sb, qk0, wkf, bkf, kT_sb, j)
                      for j in range(4)]
                q1 = [qk_item(1, xt1_sb, qk0, wqf, bqf, qT_sb, j)
                      for j in range(4)]
                v1 = [v_item(1, xt1_sb, qk0, qv) for qv in range(4)]
                dd = [delta_items(0, qk0, jq) for jq in range(4)]
                fills = [
                    vq0 + q0r + k1 + q1[0:2],              # jq0 (14)
                    q1[2:4] + v1 + dd[0],                  # jq1 (10)
                    dd[1],                                 # jq2
                    dd[2],                                 # jq3
                ]

                def post_jq0(jq):
                    if jq == 3:
                        for it in dd[3]:
                            it()
                        rs_go(0)
                attention(0, sp0, attp0, qk0, fills, post_jq=post_jq0)
            p2_ctx.close()    # free xt0
            p1_ctx.close()    # free xm + xt1

            # late pools reuse that SBUF
            late_ctx = ExitStack()
            w1res = late_ctx.enter_context(tc.tile_pool(name="w1res", bufs=1))
            tailp = late_ctx.enter_context(tc.tile_pool(name="tail", bufs=1))
            w1a = w1res.tile([P, MH, KK, P], BF16)
            nc.sync.dma_start(w1a[:, 0:MH // 2, :, :],
                              w1t.ap()[:, 0:MH // 2, :, :])
            nc.gpsimd.dma_start(w1a[:, MH // 2:MH, :, :],
                                w1t.ap()[:, MH // 2:MH, :, :])
            nc.sync.dma_start(
                xtok_sb[:], xtok.ap().rearrange("(tc p) c -> p tc c", p=P))
            yT = tailp.tile([P, KK, TOK], FP32)
            h2T = tailp.tile([P, KK, TOK], BF16)
            uT = tailp.tile([P, M, TOK], BF16)

            with tc.tile_pool(name="ph3l", bufs=1) as ph3l, \
                 tc.tile_pool(name="st2", bufs=2) as st2, \
                 tc.tile_pool(name="ffnl", bufs=3) as ffnl, \
                 tc.tile_pool(name="ffno", bufs=2) as ffno:

                def ph3_prep_items(b, stpool):
                    """y = x + rs/2048; per-channel (sum, sumsq) partials via
                    ones-column matmuls on token-major y -> AllGather."""
                    y2 = [st2.tile([P, C], BF16, tag=f"y2{j}", bufs=1,
                                   name=f"y2_{b}{j}") for j in range(2)]

                    def f1():
                        dtok = ph3l.tile([P, 2, C], F8, tag="dtok")
                        nc.gpsimd.dma_start(
                            dtok[:],
                            rs_out[b].rearrange("(j p) c -> p j c", p=P))
                        for j in range(2):
                            nc.gpsimd.scalar_tensor_tensor(
                                out=xtok_sb[:, b * 2 + j, :],
                                in0=dtok[:, j, :], scalar=1.0 / 2048,
                                in1=xtok_sb[:, b * 2 + j, :],
                                op0=ALU.mult, op1=ALU.add)
                            nc.vector.tensor_mul(
                                y2[j][:], xtok_sb[:, b * 2 + j, :],
                                xtok_sb[:, b * 2 + j, :])

                    def f2():
                        stps_f = stpool.tile([P, 512], FP32, tag="qk",
                                             name="stps")
                        stps = stps_f[:, 0:4 * KK]
                        for cc in range(KK):
                            for j in range(2):
                                nc.tensor.matmul(
                                    stps[:, 4 * cc + j:4 * cc + j + 1],
                                    lhsT=xtok_sb[:, b * 2 + j,
                                                 cc * P:(cc + 1) * P],
                                    rhs=onesc_f[:], start=True, stop=True)
                                nc.tensor.matmul(
                                    stps[:, 4 * cc + 2 + j:4 * cc + 3 + j],
                                    lhsT=y2[j][:, cc * P:(cc + 1) * P],
                                    rhs=onesc_b[:], start=True, stop=True)
                        sts = st2.tile([P, 4 * KK], FP32, tag="sts")
                        nc.vector.tensor_copy(sts[:], stps[:])
                        st = st2.tile([P, 2 * KK], FP32, tag="st")
                        sv = sts.rearrange("p (k j) -> p k j", j=2)
                        nc.vector.tensor_add(st[:], sv[:, :, 0], sv[:, :, 1])
                        nc.scalar.dma_start(ag_in[b][:], st[:])
                        nc.gpsimd.collective_compute(
                            "AllGather", ALU.bypass, replica_groups=RG,
                            ins=[ag_in[b].opt()], outs=[ag_out[b].opt()])
                    return [f1, f2]

                def ph3_prep(b, stpool):
                    for f in ph3_prep_items(b, stpool):
                        f()

                def ph3_transpose_items(b, tpp):
                    items = []
                    for j in range(2):
                        for cc in range(KK):
                            def ft(j=j, cc=cc):
                                tp_f = tpp.tile([P, 512], FP32, tag="qk",
                                                name="tp")
                                tp = tp_f[:, 0:P]
                                nc.tensor.transpose(
                                    tp,
                                    xtok_sb[:, b * 2 + j,
                                            cc * P:(cc + 1) * P],
                                    ident[:])
                                nc.vector.tensor_copy(
                                    yT[:, cc, b * TB + j * P:
                                       b * TB + (j + 1) * P], tp)
                            items.append(ft)
                    return items

                def ph3_ab2(b):
                    stg = st2.tile([P, NCORE, 2 * KK], FP32, tag="stg")
                    nc.gpsimd.dma_start(
                        stg[:], ag_out[b].rearrange("(r p) s -> p r s", p=P))
                    for step in (4, 2, 1):
                        nc.vector.tensor_add(
                            stg[:, 0:step, :], stg[:, 0:step, :],
                            stg[:, step:2 * step, :])
                    stf = stg[:, 0, :].rearrange("p (k s) -> p k s", s=2)
                    mean2 = st2.tile([P, KK], FP32, tag="mean2")
                    nc.vector.tensor_scalar_mul(mean2[:], stf[:, :, 0],
                                                1.0 / T)
                    ss2 = st2.tile([P, KK], FP32, tag="ss2")
                    nc.vector.tensor_mul(ss2[:], stf[:, :, 0], stf[:, :, 0])
                    var2 = st2.tile([P, KK], FP32, tag="var2")
                    nc.vector.scalar_tensor_tensor(
                        out=var2[:], in0=ss2[:], scalar=-1.0 / T,
                        in1=stf[:, :, 1], op0=ALU.mult, op1=ALU.add)
                    nc.vector.tensor_scalar_mul(var2[:], var2[:],
                                                1.0 / (T - 1))
                    den2 = st2.tile([P, KK], FP32, tag="den2")
                    nc.scalar.sqrt(den2[:], var2[:])
                    nc.vector.tensor_scalar_add(den2[:], den2[:], EPS)
                    rden2 = st2.tile([P, KK], FP32, tag="rden2")
                    nc.vector.reciprocal(rden2[:], den2[:])
                    A2 = st2.tile([P, KK], FP32, tag="A2")
                    nc.vector.tensor_mul(A2[:], g2_sb[:], rden2[:])
                    mA2 = st2.tile([P, KK], FP32, tag="mA2")
                    nc.vector.tensor_mul(mA2[:], mean2[:], A2[:])
                    B2 = st2.tile([P, KK], FP32, tag="B2")
                    nc.vector.tensor_sub(B2[:], be2_sb[:], mA2[:])
                    return A2, B2

                def ph3_h2(b, ab2):
                    A2, B2 = ab2
                    for kk in range(KK):
                        nc.vector.tensor_scalar(
                            out=h2T[:, kk, b * TB:(b + 1) * TB],
                            in0=yT[:, kk, b * TB:(b + 1) * TB],
                            scalar1=A2[:, kk:kk + 1],
                            scalar2=B2[:, kk:kk + 1],
                            op0=ALU.mult, op1=ALU.add)

                def ph3_finish(b):
                    ph3_h2(b, ph3_ab2(b))

                def ffn_w1_items(b, up, relu_pool=False):
                    items = []
                    for m in range(M):
                        def fm(m=m):
                            if m < MH:
                                w1_sl = w1a[:, m, :, :]
                            else:
                                w1_t = ffnl.tile([P, KK, P], BF16, tag="w1",
                                                 bufs=3)
                                nc.sync.dma_start(w1_t[:],
                                                  w1t.ap()[:, m, :, :])
                                w1_sl = w1_t[:]
                            ups = up.tile([P, TB], FP32, tag="qk", name="u")
                            for kk in range(KK):
                                nc.tensor.matmul(
                                    ups[:], lhsT=w1_sl[:, kk, :],
                                    rhs=h2T[:, kk, b * TB:(b + 1) * TB],
                                    start=(kk == 0), stop=(kk == KK - 1))
                            if relu_pool:
                                # keep ACT free for attention exps
                                nc.gpsimd.tensor_scalar(
                                    out=uT[:, m, b * TB:(b + 1) * TB],
                                    in0=ups[:], scalar1=b1_sb[:, m:m + 1],
                                    scalar2=0.0, op0=ALU.add, op1=ALU.max)
                            else:
                                nc.scalar.activation(
                                    uT[:, m, b * TB:(b + 1) * TB], ups[:],
                                    AF.Relu, bias=b1_sb[:, m:m + 1],
                                    scale=1.0)
                        items.append(fm)
                    return items

                def ffn_w2(b, zp, mid_cb=None, mid_q=M // 2):
                    zt = [zp.tile([P, C], FP32, tag="z", name=f"z{b}{j}")
                          for j in range(2)]
                    for q in range(M):
                        if q == mid_q and mid_cb is not None:
                            mid_cb()
                        w2_sl = ffnl.tile([P, C], BF16, tag="w2", bufs=3)
                        nc.sync.dma_start(w2_sl[:], w2t.ap()[:, q, :])
                        for j in range(2):
                            for nh in range(2):
                                nc.tensor.matmul(
                                    zt[j][:, nh * 512:(nh + 1) * 512],
                                    lhsT=uT[:, q, b * TB + j * P:
                                            b * TB + (j + 1) * P],
                                    rhs=w2_sl[:, nh * 512:(nh + 1) * 512],
                                    start=(q == 0), stop=(q == M - 1))
                    for j in range(2):
                        tc_i = b * 2 + j
                        o_sb = ffno.tile([P, C], FP32, tag="o", bufs=2)
                        nc.vector.tensor_add(o_sb[:], zt[j][:],
                                             xtok_sb[:, tc_i, :])
                        nc.sync.dma_start(
                            out.ap()[tc_i * P:(tc_i + 1) * P, :], o_sb[:])

                # ========== phase C: attention(1) + ph3(0) fill ==========
                with tc.tile_pool(name="sp1", bufs=2, space="PSUM") as sp1, \
                     tc.tile_pool(name="qk1", bufs=2, space="PSUM") as qk1, \
                     tc.tile_pool(name="attp1", bufs=2, space="PSUM") as attp1:
                    pits = ph3_prep_items(0, qk1)
                    tps = ph3_transpose_items(0, qk1)
                    ddc = [delta_items(1, qk1, jq) for jq in range(4)]
                    w1_0 = ffn_w1_items(0, qk1, relu_pool=True)
                    ab2_box = []

                    def fab2():
                        ab2_box.append(ph3_ab2(0))

                    def fh2():
                        ph3_h2(0, ab2_box[0])
                    # f1 needs RS(0) done (~22us after phase-B end): mid-jq1;
                    # f2 (stats + AG2(0) issue) right after; transposes once
                    # y(0) lands; ab2/h2(0) after AG2(0) (~mid-jq2); W1(0)
                    # m-blocks (gpsimd relu, ACT stays free) fill jq3.
                    fill_c = [
                        [None] * 14,                                   # jq0
                        ddc[0] + [None, pits[0], None, pits[1]]
                        + tps[0:6],                                    # jq1
                        tps[6:16] + [fab2, fh2] + ddc[1][0:2],         # jq2
                        ddc[1][2:4] + ddc[2] + w1_0[0:8],              # jq3
                    ]

                    def post_jq1(jq):
                        if jq == 3:
                            for it in ddc[3]:
                                it()
                            rs_go(1)
                    attention(1, sp1, attp1, qk1, fill_c, post_jq=post_jq1)
                    for it in w1_0[8:]:
                        it()

                # ================= phase D: FFN + ph3(1) =================
                with tc.tile_pool(name="tpp2b", bufs=2, space="PSUM") as t2b, \
                     tc.tile_pool(name="zp0", bufs=2, space="PSUM") as zp0:
                    def mid():
                        ph3_prep(1, t2b)
                        for it in ph3_transpose_items(1, t2b):
                            it()
                    ffn_w2(0, zp0, mid_cb=mid, mid_q=8)
                    ph3_finish(1)
                with tc.tile_pool(name="ffp1", bufs=2, space="PSUM") as up1:
                    for it in ffn_w1_items(1, up1):
                        it()
                with tc.tile_pool(name="zp1", bufs=2, space="PSUM") as zp1:
                    ffn_w2(1, zp1)
                if DBG:
                    nc.sync.dma_start(dbg_qT.ap(), qT_sb[:])
                    nc.sync.dma_start(dbg_kT.ap(), kT_sb[:])
                    nc.sync.dma_start(dbg_attnT.ap(), attnT[:])
                    nc.sync.dma_start(dbg_y.ap(), xtok_sb[:])
                    nc.sync.dma_start(dbg_h2.ap(), h2T[:])
                    nc.sync.dma_start(dbg_ab.ap(), ab_sb[:])
            late_ctx.close()

    nc.compile()
    return nc


def prep_inputs(x, Wq, bq, Wk, bk, Wv, bv, Wo, bo, W1, b1, W2, b2,
                gamma1, beta1, gamma2, beta2):
    bf = ml_dtypes.bfloat16
    f8 = ml_dtypes.float8_e4m3
    xf = np.asarray(x, np.float32).reshape(TN, C)
    xt_full = np.ascontiguousarray(xf.T * 16.0).astype(f8)       # [C, TN]
    w1_full = np.ascontiguousarray(
        np.asarray(W1, np.float32).reshape(KK, P, M, P)
        .transpose(1, 2, 0, 3)).astype(bf)                       # [P, M, KK, P]
    w2_full = np.ascontiguousarray(
        np.asarray(W2, np.float32).reshape(M, P, C)
        .transpose(1, 0, 2)).astype(bf)                          # [P, M, C]
    b1_t = np.ascontiguousarray(b1.reshape(M, P).T).astype(np.float32)
    g2t = np.ascontiguousarray(gamma2.reshape(KK, P).T).astype(np.float32)
    be2t = np.ascontiguousarray(beta2.reshape(KK, P).T).astype(np.float32)
    bob2 = (np.asarray(bo, np.float32) + np.asarray(b2, np.float32))[None, :]

    in_maps = []
    for i in range(NCORE):
        ci = slice(P * i, P * (i + 1))
        hA, hB = 2 * i, 2 * i + 1

        def tile_km(wcat):  # [C, 128] -> [p, kk, m], 16x scale
            return np.ascontiguousarray(
                (wcat * 16.0).reshape(KK, P, P).transpose(1, 0, 2)).astype(bf)

        wq_cat = np.concatenate([Wq[hA], Wq[hB]], axis=1)
        wk_cat = np.concatenate([Wk[hA], Wk[hB]], axis=1)
        wv_cat = np.concatenate([Wv[hA], Wv[hB]], axis=1)
        xtok_i = np.concatenate(
            [xf[i * TB:(i + 1) * TB], xf[T + i * TB:T + (i + 1) * TB]],
            axis=0) + bob2
        in_maps.append({
            "xt": xt_full,
            "xmine": np.ascontiguousarray(xt_full[ci]),
            "xtok": np.ascontiguousarray(xtok_i.astype(np.float32)),
            "wq": tile_km(np.asarray(wq_cat, np.float32)),
            "wk": tile_km(np.asarray(wk_cat, np.float32)),
            "wv": tile_km(np.asarray(wv_cat, np.float32)),
            "wor": np.ascontiguousarray(
                np.asarray(Wo, np.float32)[ci] * 256.0).astype(bf),
            "w1t": w1_full,
            "w2t": w2_full,
            "bq2": (np.concatenate([bq[hA], bq[hB]])[:, None]
                    * 256.0).astype(np.float32),
            "bk2": (np.concatenate([bk[hA], bk[hB]])[:, None]
                    * 256.0).astype(np.float32),
            "bv2": (np.concatenate([bv[hA], bv[hB]])[:, None]
                    * 8.0).astype(np.float32),
            "b1t": b1_t,
            "g1": gamma1[ci][:, None].astype(np.float32),
            "be1": beta1[ci][:, None].astype(np.float32),
            "g2f": g2t,
            "be2f": be2t,
        })
    return in_maps


def kernel(**inputs):
    inputs = {k: np.asarray(v) for k, v in inputs.items()}
    if "nc" not in _cache:
        _cache["nc"] = build()
    nc = _cache["nc"]
    in_maps = prep_inputs(**inputs)
    res = bass_utils.run_bass_kernel_spmd(nc, in_maps,
                                          core_ids=list(range(NCORE)))
    outf = np.zeros((TN, C), np.float32)
    for i in range(NCORE):
        o = res.results[i]["out"]
        outf[i * TB:(i + 1) * TB] = o[0:TB]
        outf[T + i * TB:T + (i + 1) * TB] = o[TB:TOK]
    return outf.reshape(B, T, C).astype(np.float32)


# revision 24
# speedup vs baseline: 1.2981x; 1.1013x over previous
"""Trainium2 Bass kernel for nn_Encoder (pre-norm transformer block, LN over
sequence axis) distributed over 8 NeuronCores.

v2: fp8e4m3 DoubleRow matmuls for QKV and attn@V (2x PE rate), merged LN1
stats AllGather, 2048-wide exp, fp8 ReduceScatter, b2/bo folded into xtok.

Scales: x*16 (fp8), W{q,k,v}*16 -> q,k at 256x (bf16), v/32 -> vq at 8x
(fp8), exp scale /65536, p unscaled (fp8), attnT 8x (bf16), Wo*256 host ->
delta psum 2048x -> fp8 RS, y += rs/2048. FFN stays bf16 (fp8 error too big).
"""

import numpy as np
import ml_dtypes
from contextlib import ExitStack

from concourse import bacc, bass_utils
import concourse.bass as bass
import concourse.tile as tile
import concourse.mybir as mybir
from concourse.masks import make_identity

FP32 = mybir.dt.float32
BF16 = mybir.dt.bfloat16
F8 = mybir.dt.float8e4
AF = mybir.ActivationFunctionType
ALU = mybir.AluOpType
AX = mybir.AxisListType
DRM = mybir.MatmulPerfMode.DoubleRow

B, T, C, H, HS = 2, 2048, 1024, 16, 64
NCORE, P = 8, 128
TN = B * T            # 4096 flat tokens
TOK = TN // NCORE     # 512 tokens per core (256 per batch)
TB = TOK // B         # 256 tokens per batch per core
F = 4 * C             # 4096
KK = C // P           # 8 k-tiles over C
M = F // P            # 32 m-blocks over F
MH = M // 2           # resident half of W1
TQ = 1024             # q-column block per attention region (T // 2)
EPS = 1e-5
SC_EXP = float(HS) ** -0.5 / 65536.0
RG = [list(range(NCORE))]

_cache = {}


def build():
    nc = bacc.Bacc("TRN2", target_bir_lowering=False, debug=False,
                   num_devices=NCORE)

    def EIN(name, shape, dtype):
        return nc.dram_tensor(name, shape, dtype, kind="ExternalInput")

    xt = EIN("xt", [C, TN], F8)            # 16*x^T full (replicated)
    xmine = EIN("xmine", [P, TN], F8)      # my 128 channels of 16*x^T
    xtok = EIN("xtok", [TOK, C], FP32)     # my token rows, +bo+b2 folded in
    wq = EIN("wq", [P, KK, P], BF16)       # 16*Wq cat(2 heads) tiled [p,kk,m]
    wk = EIN("wk", [P, KK, P], BF16)
    wv = EIN("wv", [P, KK, P], BF16)
    wor = EIN("wor", [P, C], BF16)         # 256*Wo rows for my heads
    w1t = EIN("w1t", [P, M, KK, P], BF16)  # [p(c in kk), m, kk, mcol]
    w2t = EIN("w2t", [P, M, C], BF16)      # [p(f in q), q, n]
    bq2 = EIN("bq2", [P, 1], FP32)         # 256*bq
    bk2 = EIN("bk2", [P, 1], FP32)         # 256*bk
    bv2 = EIN("bv2", [P, 1], FP32)         # 8*bv
    b1t = EIN("b1t", [P, M], FP32)         # [p, m]
    g1 = EIN("g1", [P, 1], FP32)           # LN1 gamma/beta for my 128 chans
    be1 = EIN("be1", [P, 1], FP32)
    g2f = EIN("g2f", [P, KK], FP32)        # LN2 gamma/beta, all chans (p, kk)
    be2f = EIN("be2f", [P, KK], FP32)
    out = nc.dram_tensor("out", [TOK, C], FP32, kind="ExternalOutput")
    DBG = _cache.get("debug", False)
    if DBG:
        dbg_qT = nc.dram_tensor("dbg_qT", [P, B, T], BF16,
                                kind="ExternalOutput")
        dbg_kT = nc.dram_tensor("dbg_kT", [P, B, T], BF16,
                                kind="ExternalOutput")
        dbg_attnT = nc.dram_tensor("dbg_attnT", [P, TN], BF16,
                                   kind="ExternalOutput")
        dbg_y = nc.dram_tensor("dbg_y", [P, 4, C], FP32,
                               kind="ExternalOutput")
        dbg_h2 = nc.dram_tensor("dbg_h2", [P, KK, TOK], BF16,
                                kind="ExternalOutput")
        dbg_ab = nc.dram_tensor("dbg_ab", [P, KK, 4], FP32,
                                kind="ExternalOutput")

    with tile.TileContext(nc) as tc, ExitStack() as ctx:
        const = ctx.enter_context(tc.tile_pool(name="const", bufs=1))
        dram = ctx.enter_context(tc.tile_pool(name="dram", bufs=1, space="DRAM"))
        persist = ctx.enter_context(tc.tile_pool(name="acts", bufs=1))

        # ---------------- DRAM comm tiles ----------------
        ab_in = dram.tile([P, 4], FP32, name="abi")
        ab_out = dram.tile([NCORE * P, 4], FP32, name="abo")
        rs_in = [dram.tile([T, C], F8, name=f"rsi{b}") for b in range(B)]
        rs_out = [dram.tile([TB, C], F8, name=f"rso{b}") for b in range(B)]
        ag_in = [dram.tile([P, 2 * KK], FP32, name=f"agi{b}") for b in range(B)]
        ag_out = [dram.tile([NCORE * P, 2 * KK], FP32, name=f"ago{b}")
                  for b in range(B)]

        with tc.tile_pool(name="attn_acts", bufs=1) as acts, \
             tc.tile_pool(name="ph2l", bufs=4) as ph2l, \
             tc.tile_pool(name="p8p", bufs=3) as p8p, \
             tc.tile_pool(name="dstg", bufs=3) as dstg:
            qT_sb = acts.tile([P, B, T], BF16)
            kT_sb = acts.tile([P, B, T], BF16)
            # v in fp8, 8x scale: per (b,hd,pair,par): [0:32]=v/32 lo,
            # [32]=ones, [33:65]=v/32 hi, [65] unused
            vq = acts.tile([P, B, 2, KK, 2, 66], F8)
            attnT = acts.tile([P, TN], BF16)

            p1_ctx = ExitStack()
            p1 = p1_ctx.enter_context(tc.tile_pool(name="p1", bufs=1))
            xm_sb = p1.tile([P, TN], F8)
            nc.sync.dma_start(xm_sb[:], xmine.ap())
            xt1_sb = p1.tile([P, KK, T], F8)

            p2_ctx = ExitStack()
            p2 = p2_ctx.enter_context(tc.tile_pool(name="p2", bufs=1))
            xt0_sb = p2.tile([P, KK, T], F8)
            src_v = xt.ap().rearrange("(kk p) n -> p kk n", p=P)
            for kk in range(2):
                nc.gpsimd.dma_start(xt0_sb[:, kk, :], src_v[:, kk, 0:T])

            ident = const.tile([P, P], FP32)
            make_identity(nc, ident)
            ones1 = const.tile([1, P], FP32)
            nc.vector.memset(ones1[:], 1.0)
            onesc_f = const.tile([P, 1], FP32)
            nc.vector.memset(onesc_f[:], 1.0)
            onesc_b = const.tile([P, 1], BF16)
            nc.vector.memset(onesc_b[:], 1.0)

            def ldconst(t, shape, dt=FP32):
                s = const.tile(shape, dt, name=t.name + "_sb")
                nc.sync.dma_start(s[:], t.ap())
                return s

            def declconst(t, shape, dt=FP32):
                return const.tile(shape, dt, name=t.name + "_sb")

            g1_sb = ldconst(g1, [P, 1])
            be1_sb = ldconst(be1, [P, 1])
            wq_sb = declconst(wq, [P, KK, P], BF16)
            wk_sb = declconst(wk, [P, KK, P], BF16)
            wv_sb = declconst(wv, [P, KK, P], BF16)
            wor_sb = declconst(wor, [P, C], BF16)
            bq_sb = declconst(bq2, [P, 1])
            bk_sb = declconst(bk2, [P, 1])
            bv_sb = declconst(bv2, [P, 1])
            b1_sb = declconst(b1t, [P, M])
            g2_sb = declconst(g2f, [P, KK])
            be2_sb = declconst(be2f, [P, KK])

            # long-lived activations
            xtok_sb = persist.tile([P, B * 2, C], FP32)  # my tokens; becomes y
            wqf = [persist.tile([P, KK, P], F8, name=f"wqf{b}")
                   for b in range(B)]
            wkf = [persist.tile([P, KK, P], F8, name=f"wkf{b}")
                   for b in range(B)]
            wvf = [persist.tile([P, KK, P], F8, name=f"wvf{b}")
                   for b in range(B)]
            bqf = [persist.tile([P, 1], FP32, name=f"bqf{b}") for b in range(B)]
            bkf = [persist.tile([P, 1], FP32, name=f"bkf{b}") for b in range(B)]
            cvec = [persist.tile([P, 1], FP32, name=f"cvec{b}")
                    for b in range(B)]
            ab_sb = persist.tile([P, KK, 4], FP32)
            bb_sb = persist.tile([P, KK, 2], BF16)
            ab_loc = persist.tile([P, 4], FP32)

            def ln_stats(b, stats):
                """(A, 16B) for my 128 chans of batch b -> ab_loc[:, 2b:2b+2].
                Input is 16*x in fp8: s1 = 16*sum, s2 = 256*sumsq; work in
                scaled units (mean_s=16*mean, den_s=16*sqrt(var))."""
                n = T
                eng = nc.vector if b == 0 else nc.gpsimd
                xsrc = xm_sb[:, b * T:(b + 1) * T]
                s1 = stats.tile([P, 1], FP32, tag=f"s1{b}")
                s2 = stats.tile([P, 1], FP32, tag=f"s2{b}")
                scr = stats.tile([P, n], FP32, tag=f"scr{b}", bufs=1)
                scr2 = stats.tile([P, n], FP32, tag=f"scs{b}", bufs=1)
                eng.scalar_tensor_tensor(
                    out=scr2[:], in0=xsrc, scalar=1.0, in1=xsrc,
                    op0=ALU.mult, op1=ALU.bypass, accum_out=s1[:])
                eng.scalar_tensor_tensor(
                    out=scr[:], in0=xsrc, scalar=1.0, in1=xsrc,
                    op0=ALU.mult, op1=ALU.mult, accum_out=s2[:])
                mean = stats.tile([P, 1], FP32, tag=f"mean{b}")
                nc.vector.tensor_scalar_mul(mean[:], s1[:], 1.0 / n)
                ss = stats.tile([P, 1], FP32, tag=f"ss{b}")
                nc.vector.tensor_mul(ss[:], s1[:], s1[:])
                var = stats.tile([P, 1], FP32, tag=f"var{b}")
                nc.vector.scalar_tensor_tensor(
                    out=var[:], in0=ss[:], scalar=-1.0 / n, in1=s2[:],
                    op0=ALU.mult, op1=ALU.add)
                nc.vector.tensor_scalar_mul(var[:], var[:], 1.0 / (n - 1))
                den = stats.tile([P, 1], FP32, tag=f"den{b}")
                nc.scalar.sqrt(den[:], var[:])
                nc.vector.tensor_scalar_add(den[:], den[:], 16.0 * EPS)
                rden = stats.tile([P, 1], FP32, tag=f"rden{b}")
                nc.vector.reciprocal(rden[:], den[:])
                # A = 16*g1/(den_s+16eps) (=A_true); bb = 16*be1 - mean_s*A
                ga = stats.tile([P, 1], FP32, tag=f"ga{b}")
                nc.vector.tensor_scalar_mul(ga[:], g1_sb[:], 16.0)
                nc.vector.tensor_mul(ab_loc[:, 2 * b:2 * b + 1], ga[:],
                                     rden[:])
                mA = stats.tile([P, 1], FP32, tag=f"mA{b}")
                nc.vector.tensor_mul(mA[:], mean[:],
                                     ab_loc[:, 2 * b:2 * b + 1])
                be16 = stats.tile([P, 1], FP32, tag=f"be{b}")
                nc.vector.tensor_scalar_mul(be16[:], be1_sb[:], 16.0)
                nc.vector.tensor_sub(ab_loc[:, 2 * b + 1:2 * b + 2],
                                     be16[:], mA[:])

            def fold(b, foldp):
                for wbase, wf in ((wq_sb, wqf), (wk_sb, wkf), (wv_sb, wvf)):
                    for kk in range(KK):
                        eng = nc.vector if kk % 2 == 0 else nc.gpsimd
                        eng.tensor_scalar_mul(
                            wf[b][:, kk, :], wbase[:, kk, :],
                            ab_sb[:, kk, 2 * b:2 * b + 1])
                for wbase, bias, dst, dsc in ((wq_sb, bq_sb, bqf, 1.0),
                                              (wk_sb, bk_sb, bkf, 1.0),
                                              (wv_sb, bv_sb, cvec, 1.0 / 32)):
                    ps = foldp.tile([P, 1], FP32, tag="bf")
                    for kk in range(KK):
                        nc.tensor.matmul(
                            ps[:], lhsT=wbase[:, kk, :],
                            rhs=bb_sb[:, kk, b:b + 1],
                            start=(kk == 0), stop=(kk == KK - 1))
                    if dsc == 1.0:
                        nc.vector.tensor_add(dst[b][:], ps[:], bias[:])
                    else:
                        nc.vector.scalar_tensor_tensor(
                            out=dst[b][:], in0=ps[:], scalar=dsc,
                            in1=bias[:], op0=ALU.mult, op1=ALU.add)

            def qk_item(b, xt_src, pool, wf, bias, dst, j):
                def fq():
                    ps = pool.tile([P, 512], FP32, tag="qk", name="qkps")
                    for h in range(2):
                        for kp in range(KK // 2):
                            nc.tensor.matmul(
                                ps[h * 64:(h + 1) * 64, :],
                                lhsT=wf[b][:, 2 * kp:2 * kp + 2,
                                           h * 64:(h + 1) * 64],
                                rhs=xt_src[:, 2 * kp:2 * kp + 2,
                                           j * 512:(j + 1) * 512],
                                start=(kp == 0), stop=(kp == KK // 2 - 1),
                                perf_mode=DRM)
                    nc.vector.tensor_scalar_add(
                        dst[:, b, j * 512:(j + 1) * 512], ps[:], bias[b][:])
                return fq

            def v_item(b, xt_src, pool, qv):
                """One vq k-chunk QUAD (pairs 2qv, 2qv+1; tokens 4qv*128..)."""
                def fv():
                    for sub in range(4):
                        tt = 4 * qv + sub
                        vps_f = pool.tile([P, 512], FP32, tag="qk",
                                          name="vps")
                        vps = vps_f[:, 0:P]
                        for h in range(2):
                            for kp in range(KK // 2):
                                nc.tensor.matmul(
                                    vps[h * 64:(h + 1) * 64, :],
                                    lhsT=xt_src[:, 2 * kp:2 * kp + 2,
                                                tt * P + h * 64:
                                                tt * P + (h + 1) * 64],
                                    rhs=wvf[b][:, 2 * kp:2 * kp + 2, :],
                                    start=(kp == 0),
                                    stop=(kp == KK // 2 - 1),
                                    perf_mode=DRM)
                        # vchan c=hd*64+half*32+i -> vq[..,hd,pr,par,half*33+i]
                        dst = vq[:, b, :, 2 * qv + sub // 2, sub % 2,
                                 :].rearrange(
                            "p h (w x) -> p h w x", x=33)[:, :, :, 0:32]
                        src = vps.rearrange("p (h w x) -> p h w x", h=2, w=2)
                        nc.vector.tensor_scalar_mul(dst, src, 1.0 / 32)
                return fv

            def attention(b, sp, attp, qk, fill_jq, post_jq=None):
                """Regions (jq, hd) of 512 q-cols; 1024-wide exp per k-pair,
                double-buffered scores psum; DoubleRow attn@V accumulates
                both v-halves in ONE psum bank (A rows 0:33 with denom,
                B rows 64:96 pre-zeroed, start-group checks skipped);
                normalize deferred into the next region's first slot."""
                pend = [None]

                def norm_epilogue(att, hd, jq):
                    def run():
                        rden = ph2l.tile([1, 512], FP32, tag="rden", bufs=2)
                        nc.vector.reciprocal(rden[:], att[32:33, :])
                        rdf = qk.tile([P, 512], FP32, tag="qk", name="rdf")
                        nc.tensor.matmul(rdf[0:64, :], lhsT=ones1[:, 0:64],
                                         rhs=rden[:], start=True, stop=True)
                        base = b * T + jq * 512
                        nc.vector.tensor_mul(
                            attnT[hd * 64:hd * 64 + 32, base:base + 512],
                            att[0:32, :], rdf[0:32, :])
                        nc.vector.tensor_mul(
                            attnT[hd * 64 + 32:hd * 64 + 64,
                                  base:base + 512],
                            att[64:96, :], rdf[32:64, :])
                        if hd == 1:
                            nc.vector.tensor_scalar_add(
                                attnT[:, base:base + 512],
                                attnT[:, base:base + 512], cvec[b][:])
                            if post_jq is not None:
                                post_jq(jq)
                    return run

                for jq in range(4):
                    fill = fill_jq[jq] or []
                    for hd in range(2):
                        att = attp.tile([P, 512], FP32, tag="att",
                                        name=f"att{b}{jq}{hd}")
                        for pr in range(KK):
                            s_ps = sp.tile([P, 2, 512], FP32, tag="s")
                            for par in range(2):
                                k0 = pr * 2 + par
                                nc.tensor.matmul(
                                    s_ps[:, par, :],
                                    lhsT=kT_sb[hd * 64:(hd + 1) * 64, b,
                                               k0 * P:(k0 + 1) * P],
                                    rhs=qT_sb[hd * 64:(hd + 1) * 64, b,
                                              jq * 512:(jq + 1) * 512],
                                    start=True, stop=True)
                            p8t = p8p.tile([P, 2, 512], F8, tag="p8")
                            nc.scalar.activation(p8t[:], s_ps[:], AF.Exp,
                                                 scale=SC_EXP)
                            nc.tensor.matmul(
                                att[0:33, :],
                                lhsT=vq[:, b, hd, pr, :, 0:33],
                                rhs=p8t[:], start=(pr == 0),
                                stop=(pr == KK - 1), perf_mode=DRM)
                            # B-half rides the same bank: pending-zero map is
                            # partition-blind, so A pr0 (mark+clear 33 rows)
                            # then B pr0 (re-mark+clear its 32 rows) leaves
                            # both accumulating cleanly; started-map checks
                            # must be skipped.
                            nc.tensor.matmul(
                                att[64:96, :],
                                lhsT=vq[:, b, hd, pr, :, 33:65],
                                rhs=p8t[:], start=(pr == 0),
                                stop=(pr == KK - 1),
                                skip_group_check=True, perf_mode=DRM)
                            if pend[0] is not None:
                                pend[0]()
                                pend[0] = None
                            elif fill:
                                it = fill.pop(0)
                                if it is not None:
                                    it()
                        pend[0] = norm_epilogue(att, hd, jq)
                    while fill:
                        it = fill.pop(0)
                        if it is not None:
                            it()
                pend[0]()

            def delta_items(b, sp, jq):
                """4 token-chunk items of delta for q-quarter jq; fp8 out at
                2048x scale."""
                items = []
                for ci in range(4):
                    def fd(ci=ci):
                        tc_i = jq * 4 + ci
                        d_sb = dstg.tile([P, C], F8, tag="dsb", bufs=3)
                        for nh in range(2):
                            dps = sp.tile([P, 512], FP32, tag="qk",
                                          name="dps")
                            nc.tensor.matmul(
                                dps[:],
                                lhsT=attnT[:, b * T + tc_i * P:
                                           b * T + (tc_i + 1) * P],
                                rhs=wor_sb[:, nh * 512:(nh + 1) * 512],
                                start=True, stop=True)
                            sl = d_sb[:, nh * 512:(nh + 1) * 512]
                            if nh == 0:
                                nc.vector.tensor_copy(sl, dps[:])
                            else:
                                nc.gpsimd.tensor_copy(sl, dps[:])
                        nc.sync.dma_start(
                            rs_in[b][tc_i * P:(tc_i + 1) * P, :], d_sb[:])
                    items.append(fd)
                return items

            def rs_go(b):
                nc.gpsimd.collective_compute(
                    "ReduceScatter", ALU.add, replica_groups=RG,
                    ins=[rs_in[b].opt()], outs=[rs_out[b].opt()])

            # ================= phase A: stats, folds, QKV(0) ===============
            with tc.tile_pool(name="stats", bufs=2) as stats, \
                 tc.tile_pool(name="foldp", bufs=2, space="PSUM") as foldp, \
                 tc.tile_pool(name="qkp", bufs=4, space="PSUM") as qkp:
                ln_stats(0, stats)
                ln_stats(1, stats)
                nc.scalar.dma_start(ab_in[:], ab_loc[:])
                nc.gpsimd.collective_compute(
                    "AllGather", ALU.bypass, replica_groups=RG,
                    ins=[ab_in.opt()], outs=[ab_out.opt()])
                # bulk loads queued AFTER the tiny stats DMA so the
                # AllGather isn't stuck behind them on the DMA engines
                for kk in range(2, KK):
                    nc.sync.dma_start(xt0_sb[:, kk, :], src_v[:, kk, 0:T])
                for wsb, wt in ((wq_sb, wq), (wk_sb, wk), (wv_sb, wv)):
                    nc.sync.dma_start(wsb[:], wt.ap())
                nc.sync.dma_start(bq_sb[:], bq2.ap())
                nc.sync.dma_start(bk_sb[:], bk2.ap())
                nc.sync.dma_start(bv_sb[:], bv2.ap())
                for kk in range(KK):
                    nc.gpsimd.dma_start(xt1_sb[:, kk, :], src_v[:, kk, T:TN])
                nc.sync.dma_start(wor_sb[:], wor.ap())
                nc.sync.dma_start(b1_sb[:], b1t.ap())
                nc.sync.dma_start(g2_sb[:], g2f.ap())
                nc.sync.dma_start(be2_sb[:], be2f.ap())
                nc.sync.dma_start(
                    ab_sb[:], ab_out.rearrange("(kk p) s -> p kk s", p=P))
                nc.vector.tensor_copy(bb_sb[:, :, 0], ab_sb[:, :, 1])
                nc.vector.tensor_copy(bb_sb[:, :, 1], ab_sb[:, :, 3])
                nc.vector.memset(vq[:, :, :, :, :, 32:33], 1.0)
                fold(0, foldp)
                fold(1, foldp)
                # serial prefix of QKV(0): K all j, Q j0, V quad 0
                for j in range(4):
                    qk_item(0, xt0_sb, qkp, wkf, bkf, kT_sb, j)()
                qk_item(0, xt0_sb, qkp, wqf, bqf, qT_sb, 0)()
                v_item(0, xt0_sb, qkp, 0)()

            # ====== phase B: attention(0) + QKV(0/1) fill + delta/RS(0) ====
            with tc.tile_pool(name="sp0", bufs=2, space="PSUM") as sp0, \
                 tc.tile_pool(name="qk0", bufs=2, space="PSUM") as qk0, \
                 tc.tile_pool(name="attp0", bufs=2, space="PSUM") as attp0:
                # V quads must stay ahead of the attn pair that consumes
                # them; QKV(1) spreads over later regions interleaved with
                # delta(0) chunks. 16 pair-slots per jq (minus norm slots).
                vq0 = [v_item(0, xt0_sb, qk0, qv) for qv in range(1, 4)]
                q0r = [qk_item(0, xt0_sb, qk0, wqf, bqf, qT_sb, j)
                       for j in (1, 2, 3)]
                k1 = [qk_item(1, xt1_sb, qk0, wkf, bkf, kT_sb, j)
                      for j in range(4)]
                q1 = [qk_item(1, xt1_sb, qk0, wqf, bqf, qT_sb, j)
                      for j in range(4)]
                v1 = [v_item(1, xt1_sb, qk0, qv) for qv in range(4)]
                dd = [delta_items(0, qk0, jq) for jq in range(4)]
                fills = [
                    vq0 + q0r + k1 + q1[0:2],              # jq0 (14)
                    q1[2:4] + v1 + dd[0],                  # jq1 (10)
                    dd[1],                                 # jq2
                    dd[2],                                 # jq3
                ]

                def post_jq0(jq):
                    if jq == 3:
                        for it in dd[3]:
                            it()
                        rs_go(0)
                attention(0, sp0, attp0, qk0, fills, post_jq=post_jq0)
            p2_ctx.close()    # free xt0
            p1_ctx.close()    # free xm + xt1

            # late pools reuse that SBUF
            late_ctx = ExitStack()
            w1res = late_ctx.enter_context(tc.tile_pool(name="w1res", bufs=1))
            tailp = late_ctx.enter_context(tc.tile_pool(name="tail", bufs=1))
            w1a = w1res.tile([P, MH, KK, P], BF16)
            nc.sync.dma_start(w1a[:, 0:MH // 2, :, :],
                              w1t.ap()[:, 0:MH // 2, :, :])
            nc.gpsimd.dma_start(w1a[:, MH // 2:MH, :, :],
                                w1t.ap()[:, MH // 2:MH, :, :])
            nc.sync.dma_start(
                xtok_sb[:], xtok.ap().rearrange("(tc p) c -> p tc c", p=P))
            yT = tailp.tile([P, KK, TOK], FP32)
            h2T = tailp.tile([P, KK, TOK], BF16)
            uT = tailp.tile([P, M, TOK], BF16)

            with tc.tile_pool(name="ph3l", bufs=1) as ph3l, \
                 tc.tile_pool(name="st2", bufs=2) as st2, \
                 tc.tile_pool(name="ffnl", bufs=3) as ffnl, \
                 tc.tile_pool(name="ffno", bufs=2) as ffno:

                def ph3_prep_items(b, stpool):
                    """y = x + rs/2048; per-channel (sum, sumsq) partials via
                    ones-column matmuls on token-major y -> AllGather."""
                    y2 = [st2.tile([P, C], BF16, tag=f"y2{j}", bufs=1,
                                   name=f"y2_{b}{j}") for j in range(2)]

                    def f1():
                        dtok = ph3l.tile([P, 2, C], F8, tag="dtok")
                        nc.gpsimd.dma_start(
                            dtok[:],
                            rs_out[b].rearrange("(j p) c -> p j c", p=P))
                        for j in range(2):
                            nc.gpsimd.scalar_tensor_tensor(
                                out=xtok_sb[:, b * 2 + j, :],
                                in0=dtok[:, j, :], scalar=1.0 / 2048,
                                in1=xtok_sb[:, b * 2 + j, :],
                                op0=ALU.mult, op1=ALU.add)
                            nc.vector.tensor_mul(
                                y2[j][:], xtok_sb[:, b * 2 + j, :],
                                xtok_sb[:, b * 2 + j, :])

                    def f2():
                        stps_f = stpool.tile([P, 512], FP32, tag="qk",
                                             name="stps")
                        stps = stps_f[:, 0:4 * KK]
                        for cc in range(KK):
                            for j in range(2):
                                nc.tensor.matmul(
                                    stps[:, 4 * cc + j:4 * cc + j + 1],
                                    lhsT=xtok_sb[:, b * 2 + j,
                                                 cc * P:(cc + 1) * P],
                                    rhs=onesc_f[:], start=True, stop=True)
                                nc.tensor.matmul(
                                    stps[:, 4 * cc + 2 + j:4 * cc + 3 + j],
                                    lhsT=y2[j][:, cc * P:(cc + 1) * P],
                                    rhs=onesc_b[:], start=True, stop=True)
                        sts = st2.tile([P, 4 * KK], FP32, tag="sts")
                        nc.vector.tensor_copy(sts[:], stps[:])
                        st = st2.tile([P, 2 * KK], FP32, tag="st")
                        sv = sts.rearrange("p (k j) -> p k j", j=2)
                        nc.vector.tensor_add(st[:], sv[:, :, 0], sv[:, :, 1])
                        nc.scalar.dma_start(ag_in[b][:], st[:])
                        nc.gpsimd.collective_compute(
                            "AllGather", ALU.bypass, replica_groups=RG,
                            ins=[ag_in[b].opt()], outs=[ag_out[b].opt()])
                    return [f1, f2]

                def ph3_prep(b, stpool):
                    for f in ph3_prep_items(b, stpool):
                        f()

                def ph3_transpose_items(b, tpp):
                    items = []
                    for j in range(2):
                        for cc in range(KK):
                            def ft(j=j, cc=cc):
                                tp_f = tpp.tile([P, 512], FP32, tag="qk",
                                                name="tp")
                                tp = tp_f[:, 0:P]
                                nc.tensor.transpose(
                                    tp,
                                    xtok_sb[:, b * 2 + j,
                                            cc * P:(cc + 1) * P],
                                    ident[:])
                                nc.vector.tensor_copy(
                                    yT[:, cc, b * TB + j * P:
                                       b * TB + (j + 1) * P], tp)
                            items.append(ft)
                    return items

                def ph3_ab2(b):
                    stg = st2.tile([P, NCORE, 2 * KK], FP32, tag="stg")
                    nc.gpsimd.dma_start(
                        stg[:], ag_out[b].rearrange("(r p) s -> p r s", p=P))
                    for step in (4, 2, 1):
                        nc.vector.tensor_add(
                            stg[:, 0:step, :], stg[:, 0:step, :],
                            stg[:, step:2 * step, :])
                    stf = stg[:, 0, :].rearrange("p (k s) -> p k s", s=2)
                    mean2 = st2.tile([P, KK], FP32, tag="mean2")
                    nc.vector.tensor_scalar_mul(mean2[:], stf[:, :, 0],
                                                1.0 / T)
                    ss2 = st2.tile([P, KK], FP32, tag="ss2")
                    nc.vector.tensor_mul(ss2[:], stf[:, :, 0], stf[:, :, 0])
                    var2 = st2.tile([P, KK], FP32, tag="var2")
                    nc.vector.scalar_tensor_tensor(
                        out=var2[:], in0=ss2[:], scalar=-1.0 / T,
                        in1=stf[:, :, 1], op0=ALU.mult, op1=ALU.add)
                    nc.vector.tensor_scalar_mul(var2[:], var2[:],
                                                1.0 / (T - 1))
                    den2 = st2.tile([P, KK], FP32, tag="den2")
                    nc.scalar.sqrt(den2[:], var2[:])
                    nc.vector.tensor_scalar_add(den2[:], den2[:], EPS)
                    rden2 = st2.tile([P, KK], FP32, tag="rden2")
                    nc.vector.reciprocal(rden2[:], den2[:])
                    A2 = st2.tile([P, KK], FP32, tag="A2")
                    nc.vector.tensor_mul(A2[:], g2_sb[:], rden2[:])
                    mA2 = st2.tile([P, KK], FP32, tag="mA2")
                    nc.vector.tensor_mul(mA2[:], mean2[:], A2[:])
                    B2 = st2.tile([P, KK], FP32, tag="B2")
                    nc.vector.tensor_sub(B2[:], be2_sb[:], mA2[:])
                    return A2, B2

                def ph3_h2(b, ab2):
                    A2, B2 = ab2
                    for kk in range(KK):
                        nc.vector.tensor_scalar(
                            out=h2T[:, kk, b * TB:(b + 1) * TB],
                            in0=yT[:, kk, b * TB:(b + 1) * TB],
                            scalar1=A2[:, kk:kk + 1],
                            scalar2=B2[:, kk:kk + 1],
                            op0=ALU.mult, op1=ALU.add)

                def ph3_finish(b):
                    ph3_h2(b, ph3_ab2(b))

                def ffn_w1_items(b, up, relu_pool=False):
                    items = []
                    for m in range(M):
                        def fm(m=m):
                            if m < MH:
                                w1_sl = w1a[:, m, :, :]
                            else:
                                w1_t = ffnl.tile([P, KK, P], BF16, tag="w1",
                                                 bufs=3)
                                nc.sync.dma_start(w1_t[:],
                                                  w1t.ap()[:, m, :, :])
                                w1_sl = w1_t[:]
                            ups = up.tile([P, TB], FP32, tag="qk", name="u")
                            for kk in range(KK):
                                nc.tensor.matmul(
                                    ups[:], lhsT=w1_sl[:, kk, :],
                                    rhs=h2T[:, kk, b * TB:(b + 1) * TB],
                                    start=(kk == 0), stop=(kk == KK - 1))
                            if relu_pool:
                                # keep ACT free for attention exps
                                nc.gpsimd.tensor_scalar(
                                    out=uT[:, m, b * TB:(b + 1) * TB],
                                    in0=ups[:], scalar1=b1_sb[:, m:m + 1],
                                    scalar2=0.0, op0=ALU.add, op1=ALU.max)
                            else:
                                nc.scalar.activation(
                                    uT[:, m, b * TB:(b + 1) * TB], ups[:],
                                    AF.Relu, bias=b1_sb[:, m:m + 1],
                                    scale=1.0)
                        items.append(fm)
                    return items

                def ffn_w2(b, zp, mid_cb=None, mid_q=M // 2):
                    zt = [zp.tile([P, C], FP32, tag="z", name=f"z{b}{j}")
                          for j in range(2)]
                    for q in range(M):
                        if q == mid_q and mid_cb is not None:
                            mid_cb()
                        w2_sl = ffnl.tile([P, C], BF16, tag="w2", bufs=3)
                        nc.sync.dma_start(w2_sl[:], w2t.ap()[:, q, :])
                        for j in range(2):
                            for nh in range(2):
                                nc.tensor.matmul(
                                    zt[j][:, nh * 512:(nh + 1) * 512],
                                    lhsT=uT[:, q, b * TB + j * P:
                                            b * TB + (j + 1) * P],
                                    rhs=w2_sl[:, nh * 512:(nh + 1) * 512],
                                    start=(q == 0), stop=(q == M - 1))
                    for j in range(2):
                        tc_i = b * 2 + j
                        o_sb = ffno.tile([P, C], FP32, tag="o", bufs=2)
                        nc.vector.tensor_add(o_sb[:], zt[j][:],
                                             xtok_sb[:, tc_i, :])
                        nc.sync.dma_start(
                            out.ap()[tc_i * P:(tc_i + 1) * P, :], o_sb[:])

                # ========== phase C: attention(1) + ph3(0) fill ==========
                with tc.tile_pool(name="sp1", bufs=2, space="PSUM") as sp1, \
                     tc.tile_pool(name="qk1", bufs=2, space="PSUM") as qk1, \
                     tc.tile_pool(name="attp1", bufs=2, space="PSUM") as attp1:
                    pits = ph3_prep_items(0, qk1)
                    tps = ph3_transpose_items(0, qk1)
                    ddc = [delta_items(1, qk1, jq) for jq in range(4)]
                    w1_0 = ffn_w1_items(0, qk1, relu_pool=True)
                    ab2_box = []

                    def fab2():
                        ab2_box.append(ph3_ab2(0))

                    def fh2():
                        ph3_h2(0, ab2_box[0])
                    # f1 needs RS(0) done (~22us after phase-B end): mid-jq1;
                    # f2 (stats + AG2(0) issue) right after; transposes once
                    # y(0) lands; ab2/h2(0) after AG2(0) (~mid-jq2); W1(0)
                    # m-blocks (gpsimd relu, ACT stays free) fill jq3.
                    fill_c = [
                        [None] * 14,                                   # jq0
                        ddc[0] + [None, pits[0], None, pits[1]]
                        + tps[0:6],                                    # jq1
                        tps[6:16] + [fab2, fh2] + ddc[1][0:2],         # jq2
                        ddc[1][2:4] + ddc[2] + w1_0[0:8],              # jq3
                    ]

                    def post_jq1(jq):
                        if jq == 3:
                            for it in ddc[3]:
                                it()
                            rs_go(1)
                    attention(1, sp1, attp1, qk1, fill_c, post_jq=post_jq1)
                    # ACT is idle after the last exp: use it for these relus
                    for it in ffn_w1_items(0, qk1)[8:]:
                        it()

                # ================= phase D: FFN + ph3(1) =================
                with tc.tile_pool(name="tpp2b", bufs=2, space="PSUM") as t2b, \
                     tc.tile_pool(name="zp0", bufs=2, space="PSUM") as zp0:
                    def mid():
                        ph3_prep(1, t2b)
                        for it in ph3_transpose_items(1, t2b):
                            it()
                    ffn_w2(0, zp0, mid_cb=mid, mid_q=8)
                    ph3_finish(1)
                with tc.tile_pool(name="ffp1", bufs=2, space="PSUM") as up1:
                    for it in ffn_w1_items(1, up1):
                        it()
                with tc.tile_pool(name="zp1", bufs=2, space="PSUM") as zp1:
                    ffn_w2(1, zp1)
                if DBG:
                    nc.sync.dma_start(dbg_qT.ap(), qT_sb[:])
                    nc.sync.dma_start(dbg_kT.ap(), kT_sb[:])
                    nc.sync.dma_start(dbg_attnT.ap(), attnT[:])
                    nc.sync.dma_start(dbg_y.ap(), xtok_sb[:])
                    nc.sync.dma_start(dbg_h2.ap(), h2T[:])
                    nc.sync.dma_start(dbg_ab.ap(), ab_sb[:])
            late_ctx.close()

    nc.compile()
    return nc


def prep_inputs(x, Wq, bq, Wk, bk, Wv, bv, Wo, bo, W1, b1, W2, b2,
                gamma1, beta1, gamma2, beta2):
    bf = ml_dtypes.bfloat16
    f8 = ml_dtypes.float8_e4m3
    xf = np.asarray(x, np.float32).reshape(TN, C)
    xt_full = np.ascontiguousarray(xf.T * 16.0).astype(f8)       # [C, TN]
    w1_full = np.ascontiguousarray(
        np.asarray(W1, np.float32).reshape(KK, P, M, P)
        .transpose(1, 2, 0, 3)).astype(bf)                       # [P, M, KK, P]
    w2_full = np.ascontiguousarray(
        np.asarray(W2, np.float32).reshape(M, P, C)
        .transpose(1, 0, 2)).astype(bf)                          # [P, M, C]
    b1_t = np.ascontiguousarray(b1.reshape(M, P).T).astype(np.float32)
    g2t = np.ascontiguousarray(gamma2.reshape(KK, P).T).astype(np.float32)
    be2t = np.ascontiguousarray(beta2.reshape(KK, P).T).astype(np.float32)
    bob2 = (np.asarray(bo, np.float32) + np.asarray(b2, np.float32))[None, :]

    in_maps = []
    for i in range(NCORE):
        ci = slice(P * i, P * (i + 1))
        hA, hB = 2 * i, 2 * i + 1

        def tile_km(wcat):  # [C, 128] -> [p, kk, m], 16x scale
            return np.ascontiguousarray(
                (wcat * 16.0).reshape(KK, P, P).transpose(1, 0, 2)).astype(bf)

        wq_cat = np.concatenate([Wq[hA], Wq[hB]], axis=1)
        wk_cat = np.concatenate([Wk[hA], Wk[hB]], axis=1)
        wv_cat = np.concatenate([Wv[hA], Wv[hB]], axis=1)
        xtok_i = np.concatenate(
            [xf[i * TB:(i + 1) * TB], xf[T + i * TB:T + (i + 1) * TB]],
            axis=0) + bob2
        in_maps.append({
            "xt": xt_full,
            "xmine": np.ascontiguousarray(xt_full[ci]),
            "xtok": np.ascontiguousarray(xtok_i.astype(np.float32)),
            "wq": tile_km(np.asarray(wq_cat, np.float32)),
            "wk": tile_km(np.asarray(wk_cat, np.float32)),
            "wv": tile_km(np.asarray(wv_cat, np.float32)),
            "wor": np.ascontiguousarray(
                np.asarray(Wo, np.float32)[ci] * 256.0).astype(bf),
            "w1t": w1_full,
            "w2t": w2_full,
            "bq2": (np.concatenate([bq[hA], bq[hB]])[:, None]
                    * 256.0).astype(np.float32),
            "bk2": (np.concatenate([bk[hA], bk[hB]])[:, None]
                    * 256.0).astype(np.float32),
            "bv2": (np.concatenate([bv[hA], bv[hB]])[:, None]
                    * 8.0).astype(np.float32),
            "b1t": b1_t,
            "g1": gamma1[ci][:, None].astype(np.float32),
            "be1": beta1[ci][:, None].astype(np.float32),
            "g2f": g2t,
            "be2f": be2t,
        })
    return in_maps


def kernel(**inputs):
    inputs = {k: np.asarray(v) for k, v in inputs.items()}
    if "nc" not in _cache:
        _cache["nc"] = build()
    nc = _cache["nc"]
    in_maps = prep_inputs(**inputs)
    res = bass_utils.run_bass_kernel_spmd(nc, in_maps,
                                          core_ids=list(range(NCORE)))
    outf = np.zeros((TN, C), np.float32)
    for i in range(NCORE):
        o = res.results[i]["out"]
        outf[i * TB:(i + 1) * TB] = o[0:TB]
        outf[T + i * TB:T + (i + 1) * TB] = o[TB:TOK]
    return outf.reshape(B, T, C).astype(np.float32)
